# revision 1
# baseline (speedup 1.0000x reference)
"""DeepTDATransformer TRN2 Bass kernel: 4-core sample-parallel."""
import numpy as np
import concourse.bacc as bacc
import concourse.tile as tile
import concourse.mybir as mybir
from concourse import bass_utils

dt = mybir.dt
AF = mybir.ActivationFunctionType
ALU = mybir.AxisListType  # placeholder to avoid typo
ALU = mybir.AluOpType
AX = mybir.AxisListType
F32 = np.float32
TS = dt.float32
TR = dt.float32r

S, SP, E, H, DH, L, NCls = 1000, 1024, 256, 8, 32, 6, 2
EC = 2   # e chunks
HC = 8   # ffn hidden chunks

_uid = [0]


def _nm(p="i"):
    _uid[0] += 1
    return f"{p}{_uid[0]}"


def build_nc():
    nc = bacc.Bacc("TRN2", target_bir_lowering=False, debug=False, num_devices=4)
    d = {}

    def din(name, shape):
        d[name] = nc.dram_tensor(name, list(shape), dt.float32, kind="ExternalInput").ap()

    din("seqT5", (5, SP)); din("seqPH", (128, 40))
    din("embw1T", (5, 128)); din("embb1", (128, 1))
    din("embw2T", (128, EC * 128)); din("embb2", (128, EC))
    din("embln", (128, 2 * EC)); din("posT", (128, EC * SP))
    din("qwT", (128, L * EC * E)); din("kwT", (128, L * EC * E))
    din("vwT", (128, L * EC * E)); din("owT", (128, L * EC * E))
    din("qkvb", (128, L * 3 * EC)); din("obias", (128, L * EC))
    din("vbrow", (1, L * E))
    din("fw1T", (128, L * EC * 1024)); din("fw2T", (128, L * HC * E))
    din("fb1", (128, L * HC)); din("fb2", (128, L * EC))
    din("lng", (128, L * EC)); din("lnb", (128, L * EC))
    din("pew1T", (6, L * 128)); din("peb1", (128, L))
    din("pew2T", (128, L * E)); din("peb2", (128, L * EC))
    din("clng", (128, EC)); din("clnb", (128, EC))
    din("cw1T", (128, EC * 128)); din("cb1", (128, 1))
    din("cw2T", (128, NCls)); din("cb2", (NCls, 1))
    din("ph_law", (1, 2)); din("ph_lab", (1, 1)); din("ph_fw", (1, 1)); din("ph_db", (1, 1))
    din("tconst", (128, 8)); din("padneg", (128, 8)); din("vmask", (128, 8))
    din("iota50", (128, 50)); din("I50", (50, 50)); din("maskD50", (50, 50)); din("I4", (4, 4))
    din("I128", (128, 128)); din("ones128", (128, 1)); din("ones50", (50, 1))
    din("ones1x128", (1, 128)); din("ones1x50", (1, 50)); din("ones1x32", (1, 32))
    din("ones4", (4, 1)); din("onesEC", (EC, 1))
    din("v0", (50, 1)); din("W0", (50, 4)); din("zeros128", (128, 256))
    din("epsb", (128, 1))
    din("vmask8", (128, 64))
    out_d = nc.dram_tensor("out", [NCls, 1], dt.float32, kind="ExternalOutput").ap()

    with tile.TileContext(nc) as tc:
        with (
            tc.tile_pool(name="const", bufs=1) as cp,
            tc.tile_pool(name="wp", bufs=1) as wp,
            tc.tile_pool(name="ap_", bufs=1) as app,
            tc.tile_pool(name="sm", bufs=2) as sm,
            tc.tile_pool(name="sm4", bufs=3) as sm4,
            tc.tile_pool(name="rp", bufs=4) as rp,
            tc.tile_pool(name="psA", bufs=2, space="PSUM") as psA,
            tc.tile_pool(name="psB", bufs=1, space="PSUM") as psB,
        ):
            def c32(name, shape, nm=None):
                t = cp.tile(list(shape), TS, tag=nm or name, name=nm or name)
                nc.sync.dma_start(t[:], d[name])
                return t

            def c32r(name, shape, nm=None):
                t = cp.tile(list(shape), TR, tag=(nm or name) + "r", name=(nm or name) + "r")
                nc.gpsimd.dma_start(t[:], d[name])
                return t

            seqT5 = c32r("seqT5", (5, SP))
            seqPH = c32("seqPH", (128, 40))
            tconst = c32("tconst", (128, 8)); padneg = c32("padneg", (128, 8))
            vmask = c32("vmask", (128, 8)); iota50 = c32("iota50", (128, 50))
            I50r = c32r("I50", (50, 50)); maskD50 = c32("maskD50", (50, 50))
            I4 = c32("I4", (4, 4)); I4r = c32r("I4", (4, 4), "I4c"); I128r = c32r("I128", (128, 128))
            ones128r = c32r("ones128", (128, 1)); ones50r = c32r("ones50", (50, 1))
            o1x128r = c32r("ones1x128", (1, 128)); o1x50r = c32r("ones1x50", (1, 50))
            o1x32r = c32r("ones1x32", (1, 32)); ones4r = c32r("ones4", (4, 1))
            onesECr = c32r("onesEC", (EC, 1))
            v0 = c32r("v0", (50, 1)); W0r = c32r("W0", (50, 4))
            zeros128 = c32("zeros128", (128, 256))
            vmask8 = c32("vmask8", (128, 64))
            epsb = c32("epsb", (128, 1))
            embw1T = c32r("embw1T", (5, 128)); embb1 = c32("embb1", (128, 1))
            embw2T = c32r("embw2T", (128, EC * 128)); embb2 = c32("embb2", (128, EC))
            embln = c32("embln", (128, 2 * EC)); posT = c32("posT", (128, EC * SP))
            pew1T = c32r("pew1T", (6, L * 128)); peb1 = c32("peb1", (128, L))
            pew2T = c32r("pew2T", (128, L * E)); peb2 = c32("peb2", (128, L * EC))
            clng = c32("clng", (128, EC)); clnb = c32("clnb", (128, EC))
            cw1T = c32r("cw1T", (128, EC * 128)); cb1 = c32("cb1", (128, 1))
            cw2T = c32r("cw2T", (128, NCls)); cb2 = c32("cb2", (NCls, 1))
            law = c32r("ph_law", (1, 2)); lab = c32r("ph_lab", (1, 1))
            phfw = c32r("ph_fw", (1, 1)); phdb = c32r("ph_db", (1, 1))

            def pt(shape, tag="pj"):
                pool = psA if tag in ("pj", "sc") else psB
                return pool.tile(list(shape), TS, tag=tag, name=_nm("p"))

            def sb(shape, dtype=TS, pool=sm, tag=None):
                if tag is None:
                    fbytes = int(np.prod(shape[1:])) * 4
                    if fbytes >= 2048:
                        return sm4.tile(list(shape), dtype, tag=f"g{fbytes}", name=_nm("s"))
                    tag = _nm("t")
                elif tag in ("row1k",):
                    return rp.tile(list(shape), dtype, tag=tag, name=_nm("s"))
                return pool.tile(list(shape), dtype, tag=tag, name=_nm("s"))

            def copy(dst, src):
                nc.vector.tensor_copy(dst, src)

            MM = nc.tensor.matmul

            def MMs(out, lhsT, rhs, **kw):
                l2 = lhsT.bitcast(TS) if lhsT.dtype == TR else lhsT
                r2 = rhs.bitcast(TS) if rhs.dtype == TR else rhs
                return MM(out, l2, r2, **kw)

            # ================= PH =================
            mfeat = sb((128, 8))
            nc.vector.tensor_reduce(mfeat[:], seqPH[:].rearrange("p (c f) -> p c f", f=5),
                                    AX.X, ALU.add)
            nc.vector.tensor_scalar_mul(mfeat[:], mfeat[:], 0.2)
            p1 = pt((128, 8))
            MMs(p1[:, 0:2], o1x128r[:], law[:], start=True, stop=True)
            MMs(p1[:, 2:3], o1x128r[:], lab[:], start=True, stop=True)
            lawB = sb((128, 4))
            copy(lawB[:], p1[:, 0:4])
            scs = sb((128, 8))
            nc.vector.tensor_scalar(scs[:], tconst[:], lawB[:, 0:1], None, ALU.mult)
            tmp8 = sb((128, 8))
            nc.vector.tensor_scalar(tmp8[:], mfeat[:], lawB[:, 1:2], None, ALU.mult)
            nc.vector.tensor_add(scs[:], scs[:], tmp8[:])
            nc.vector.tensor_scalar(scs[:], scs[:], lawB[:, 2:3], None, ALU.add)
            nc.vector.tensor_add(scs[:], scs[:], padneg[:])
            scr = sb((128, 8), TR)
            copy(scr[:], scs[:])
            p2 = pt((1, 1024), tag="b")
            for c in range(8):
                MMs(p2[:, c * 128:(c + 1) * 128], scr[:, c:c + 1], I128r[:], start=True, stop=True)
            srow = sb((1, 1024), TR)
            copy(srow[:], p2[:])
            sROW = sb((128, 1024))
            for hh in range(2):
                p3 = pt((128, 512))
                MMs(p3[:], o1x128r[:], srow[:, hh * 512:(hh + 1) * 512],
                    start=True, stop=True)
                copy(sROW[:, hh * 512:(hh + 1) * 512], p3[:])
            rank = sb((128, 8))
            scratch = sb((128, 1024))
            for c in range(8):
                nc.vector.tensor_scalar(scratch[:], sROW[:], scs[:, c:c + 1], 0.0,
                                        ALU.is_gt, ALU.add, accum_out=rank[:, c:c + 1])
            ptsr = sb((128, 16), TR)
            pv = ptsr[:].rearrange("p (c two) -> p c two", two=2)
            copy(pv[:, :, 0:1], tconst[:].rearrange("p (c o) -> p c o", o=1))
            copy(pv[:, :, 1:2], mfeat[:].rearrange("p (c o) -> p c o", o=1))
            Gc = sb((128, 400), TR, tag="Gc", pool=app)
            for c in range(8):
                nc.vector.tensor_scalar(Gc[:, c * 50:(c + 1) * 50], iota50[:],
                                        rank[:, c:c + 1], None, ALU.is_equal)
            plm = pt((50, 2), tag="pj")
            plmT = pt((2, 50), tag="b")
            for c in range(8):
                MMs(plm[:], Gc[:, c * 50:(c + 1) * 50], ptsr[:, c * 2:(c + 1) * 2],
                   start=(c == 0), stop=(c == 7))
            for c in range(8):
                MMs(plmT[:], ptsr[:, c * 2:(c + 1) * 2], Gc[:, c * 50:(c + 1) * 50],
                   start=(c == 0), stop=(c == 7))
            lmT = sb((2, 50), TR)
            copy(lmT[:], plmT[:])
            pg = pt((50, 50))
            MMs(pg[:], lmT[:], lmT[:], start=True, stop=True)
            gram = sb((50, 50))
            copy(gram[:], pg[:])
            sqd = sb((50, 50))
            nc.vector.tensor_mul(sqd[:], gram[:], maskD50[:])
            sq = sb((50, 1))
            nc.vector.tensor_reduce(sq[:], sqd[:], AX.X, ALU.add)
            t1 = sb((50, 50))
            nc.vector.tensor_scalar(t1[:], gram[:], -2.0, sq[:], ALU.mult, ALU.add)
            sqr = sb((50, 1), TR)
            copy(sqr[:], sq[:])
            p4 = pt((1, 50), tag="b")
            MMs(p4[:], sqr[:], I50r[:], start=True, stop=True)
            sqrow = sb((1, 50), TR)
            copy(sqrow[:], p4[:])
            p5 = pt((50, 50), tag="b")
            MMs(p5[:], o1x50r[:], sqrow[:], start=True, stop=True)
            d2 = sb((50, 50))
            nc.vector.tensor_add(d2[:], t1[:], p5[:])
            nc.vector.tensor_scalar_max(d2[:], d2[:], 1e-30)
            lnd = sb((50, 50))
            nc.scalar.activation(lnd[:], d2[:], AF.Ln)
            distm = sb((50, 50))
            nc.scalar.activation(distm[:], lnd[:], AF.Exp, scale=0.5)
            p6 = pt((50, 2), tag="pj")
            MMs(p6[:, 0:1], o1x50r[:], phfw[:], start=True, stop=True)
            MMs(p6[:, 1:2], o1x50r[:], phdb[:], start=True, stop=True)
            fwdb = sb((50, 2))
            copy(fwdb[:], p6[:])
            nfw = sb((50, 2))
            nc.scalar.activation(nfw[:, 0:1], fwdb[:, 0:1], AF.Abs)
            nc.vector.tensor_scalar_mul(nfw[:, 1:2], fwdb[:, 1:2], -1.0)
            dists = sb((50, 50))
            nc.vector.tensor_scalar(dists[:], distm[:], nfw[:, 0:1], None, ALU.mult)
            Km = sb((50, 50))
            nc.scalar.activation(Km[:], dists[:], AF.Exp, scale=-1.0, bias=nfw[:, 1:2])
            s_r = sb((50, 1))
            nc.vector.tensor_reduce(s_r[:], Km[:], AX.X, ALU.add)
            Bm = sb((50, 50))
            nc.vector.tensor_scalar(Bm[:], maskD50[:], s_r[:], None, ALU.mult)
            nc.vector.tensor_sub(Bm[:], Bm[:], Km[:])
            nc.vector.tensor_scalar_mul(Bm[:], Bm[:], -1.0)
            D40 = sb((50, 50))
            nc.vector.tensor_scalar_mul(D40[:], maskD50[:], 40.0)
            nc.vector.tensor_add(Bm[:], Bm[:], D40[:])
            nc.vector.tensor_scalar_add(Bm[:], Bm[:], -0.8)
            Br = sb((50, 50), TR)
            copy(Br[:], Bm[:])

            def vec_norm(vr):
                pn = pt((1, 1))
                MMs(pn[:], vr[:], vr[:], start=True, stop=True)
                lnv = sb((1, 1))
                nc.scalar.activation(lnv[:], pn[:], AF.Ln)
                rs = sb((1, 1), TR)
                nc.scalar.activation(rs[:], lnv[:], AF.Exp, scale=-0.5)
                prb = pt((50, 1), tag="pj")
                MMs(prb[:], o1x50r[:], rs[:], start=True, stop=True)
                vn = sb((50, 1), TR, tag="vpow")
                nc.vector.tensor_mul(vn[:].bitcast(TS), vr[:].bitcast(TS), prb[:])
                vn2 = sb((50, 1), TR, tag="vpow")
                copy(vn2[:], vn[:].bitcast(TS))
                return vn2

            v = v0
            for it in range(12):
                pv_ = pt((50, 1))
                MMs(pv_[:], Br[:], v[:], start=True, stop=True)
                v = sb((50, 1), TR, tag="vpow")
                nc.vector.tensor_scalar_mul(v[:], pv_[:], 0.125)
                if it % 4 == 3:
                    v = vec_norm(v)
            v = vec_norm(v)
            pbv = pt((50, 1))
            MMs(pbv[:], Br[:], v[:], start=True, stop=True)
            vbvf = sb((50, 1))
            nc.vector.tensor_mul(vbvf[:], v[:].bitcast(TS), pbv[:])
            vbv = sb((50, 1), TR)
            copy(vbv[:], vbvf[:])
            pmu = pt((1, 1))
            MMs(pmu[:], vbv[:], ones50r[:], start=True, stop=True)
            mu1 = sb((1, 1))
            copy(mu1[:], pmu[:])
            pvr = pt((1, 50), tag="b")
            MMs(pvr[:], v[:], I50r[:], start=True, stop=True)
            vRow = sb((1, 50), TR)
            copy(vRow[:], pvr[:])

            def ns_orth(W, nstep):
                pg_ = pt((4, 4))
                MMs(pg_[:], W[:], W[:], start=True, stop=True)
                gd = sb((4, 4))
                nc.vector.tensor_mul(gd[:], pg_[:], I4[:])
                gdr = sb((4, 1))
                nc.vector.tensor_reduce(gdr[:], gd[:], AX.X, ALU.add)
                gdr2 = sb((4, 1), TR)
                copy(gdr2[:], gdr[:])
                ptr = pt((1, 1))
                MMs(ptr[:], gdr2[:], ones4r[:], start=True, stop=True)
                lnt = sb((1, 1))
                nc.scalar.activation(lnt[:], ptr[:], AF.Ln, scale=0.25)
                rst = sb((1, 1), TR)
                nc.scalar.activation(rst[:], lnt[:], AF.Exp, scale=-0.5)
                prb = pt((50, 1), tag="pj")
                MMs(prb[:], o1x50r[:], rst[:], start=True, stop=True)
                Wn = sb((50, 4), TR, tag="Wsub")
                nc.vector.tensor_scalar(Wn[:], W[:].bitcast(TS), prb[:], None, ALU.mult)
                W = Wn
                for _ in range(nstep):
                    pg2 = pt((4, 4))
                    MMs(pg2[:], W[:], W[:], start=True, stop=True)
                    i4h = sb((4, 4))
                    nc.vector.tensor_scalar_mul(i4h[:], I4[:], 1.5)
                    corrf = sb((4, 4))
                    nc.vector.tensor_scalar(corrf[:], pg2[:], -0.5, None, ALU.mult)
                    corr = sb((4, 4), TR)
                    nc.vector.tensor_add(corr[:], corrf[:], i4h[:])
                    pwt = pt((4, 50), tag="b")
                    MMs(pwt[:], W[:], I50r[:], start=True, stop=True)
                    WT = sb((4, 50), TR)
                    copy(WT[:], pwt[:])
                    pw2 = pt((50, 4), tag="pj")
                    MMs(pw2[:], WT[:], corr[:], start=True, stop=True)
                    W = sb((50, 4), TR, tag="Wsub")
                    copy(W[:], pw2[:])
                return W

            W = W0r
            for it in range(14):
                pw_ = pt((50, 4))
                MMs(pw_[:], Br[:], W[:], start=True, stop=True)
                Wn = sb((50, 4), TR, tag="Wsub")
                nc.vector.tensor_scalar_mul(Wn[:], pw_[:], 0.125)
                W = Wn
                pc_ = pt((1, 4))
                MMs(pc_[:], v[:], W[:], start=True, stop=True)
                cvw = sb((1, 4), TR)
                copy(cvw[:], pc_[:])
                pcor = pt((50, 4), tag="pj")
                MMs(pcor[:], vRow[:], cvw[:], start=True, stop=True)
                Wn = sb((50, 4), TR, tag="Wsub")
                nc.vector.tensor_sub(Wn[:].bitcast(TS), W[:].bitcast(TS), pcor[:])
                W2_ = sb((50, 4), TR, tag="Wsub")
                copy(W2_[:], Wn[:].bitcast(TS))
                W = W2_
                if it % 6 == 5:
                    W = ns_orth(W, 3)
            W = ns_orth(W, 6)
            pbw = pt((50, 4))
            MMs(pbw[:], Br[:], W[:], start=True, stop=True)
            BW = sb((50, 4), TR)
            copy(BW[:], pbw[:])
            ph4 = pt((4, 4))
            MMs(ph4[:], W[:], BW[:], start=True, stop=True)
            H4 = sb((4, 4))
            copy(H4[:], ph4[:])
            h4d = sb((4, 4)); h4f = sb((4, 4))
            nc.vector.tensor_mul(h4d[:], H4[:], I4[:])
            nc.vector.tensor_mul(h4f[:], H4[:], H4[:])
            rd = sb((4, 1)); rf = sb((4, 1))
            nc.vector.tensor_reduce(rd[:], h4d[:], AX.X, ALU.add)
            nc.vector.tensor_reduce(rf[:], h4f[:], AX.X, ALU.add)
            rdr = sb((4, 2), TR)
            copy(rdr[:, 0:1], rd[:]); copy(rdr[:, 1:2], rf[:])
            pst = pt((2, 1))
            MMs(pst[:], rdr[:], ones4r[:], start=True, stop=True)
            stt2 = sb((2, 1), TR)
            copy(stt2[:], pst[:])
            pstr = pt((1, 2))
            MMs(pstr[:], stt2[:], I4r[0:2, 0:2], start=True, stop=True)
            sttrow = sb((1, 2))
            copy(sttrow[:], pstr[:])
            frH0 = sttrow[0:1, 1:2]
            mean_mu = sb((1, 1))
            nc.vector.tensor_scalar_mul(mean_mu[:], sttrow[0:1, 0:1], 0.25)
            m2 = sb((1, 1))
            nc.vector.tensor_mul(m2[:], mean_mu[:], mean_mu[:])
            nc.vector.tensor_scalar_mul(m2[:], m2[:], -4.0 / 3.0)
            varq = sb((1, 1))
            nc.vector.tensor_scalar_mul(varq[:], frH0[:], 1.0 / 3.0)
            nc.vector.tensor_add(varq[:], varq[:], m2[:])
            nc.vector.tensor_scalar_max(varq[:], varq[:], 1e-30)
            lnv3 = sb((1, 1))
            nc.scalar.activation(lnv3[:], varq[:], AF.Ln)
            std_ev = sb((1, 1))
            nc.scalar.activation(std_ev[:], lnv3[:], AF.Exp, scale=0.5)
            mean_ev = sb((1, 1))
            nc.vector.tensor_scalar(mean_ev[:], mean_mu[:], -1.0, 40.0, ALU.mult, ALU.add)
            gap = sb((1, 1))
            nc.vector.tensor_scalar(gap[:], mu1[:], -1.0, 40.0, ALU.mult, ALU.add)
            pfrow = sb((1, 8))
            copy(pfrow[:], zeros128[0:1, 0:8])
            nc.vector.tensor_scalar_add(pfrow[:, 0:1], pfrow[:, 0:1], 1.0)
            nc.vector.tensor_scalar_add(pfrow[:, 3:4], pfrow[:, 3:4], 1.0 / 7.0)
            copy(pfrow[:, 2:3], gap[:])
            copy(pfrow[:, 4:5], mean_ev[:])
            copy(pfrow[:, 5:6], std_ev[:])
            pfrr = sb((1, 8), TR)
            copy(pfrr[:], pfrow[:])
            ppf = pt((8, 1))
            MMs(ppf[:], pfrr[:], o1x128r[:, 0:1], start=True, stop=True)
            pfr = sb((8, 1), TR)
            copy(pfr[:], ppf[:])
            pfr = pfr[0:6, :]

            # ts per layer
            tsB = sb((128, L), tag="tsB", pool=app)
            for l in range(L):
                ph1 = pt((128, 1))
                MMs(ph1[:], pew1T[:, l * 128:(l + 1) * 128], pfr[:], start=True, stop=True)
                h1f = sb((128, 1))
                nc.vector.tensor_scalar(h1f[:], ph1[:], peb1[:, l:l + 1], None, ALU.add)
                h1b = sb((128, 1), TR)
                nc.vector.tensor_scalar_max(h1b[:], h1f[:], 0.0)
                sig = sb((128, EC))
                for co in range(EC):
                    py = pt((128, 1))
                    MMs(py[:], pew2T[:, (l * EC + co) * 128:(l * EC + co + 1) * 128],
                       h1b[:], start=True, stop=True)
                    yb = sb((128, 1))
                    nc.vector.tensor_scalar(yb[:], py[:], peb2[:, l * EC + co:l * EC + co + 1],
                                            None, ALU.add)
                    ey = sb((128, 1))
                    nc.scalar.activation(ey[:], yb[:], AF.Exp, scale=-1.0)
                    nc.vector.tensor_scalar_add(ey[:], ey[:], 1.0)
                    nc.vector.reciprocal(sig[:, co:co + 1], ey[:])
                sigr = sb((128, EC), TR)
                copy(sigr[:], sig[:])
                pts_ = pt((EC, 1))
                MMs(pts_[:], sigr[:], ones128r[:], start=True, stop=True)
                tsum = sb((EC, 1), TR)
                copy(tsum[:], pts_[:])
                pt2 = pt((1, 1))
                MMs(pt2[:], tsum[:], onesECr[:], start=True, stop=True)
                tsv = sb((1, 1), TR)
                nc.vector.tensor_scalar_mul(tsv[:], pt2[:], float(1.0 / (256.0 * np.sqrt(32.0))))
                ptb = pt((128, 1))
                MMs(ptb[:], o1x128r[:], tsv[:], start=True, stop=True)
                copy(tsB[:, l:l + 1], ptb[:])

            # ================= embedding =================
            e1 = sb((128, SP), TR, tag="ln_x2", pool=app)
            for th in range(2):
                pe_ = pt((128, 512))
                MM(pe_[:, 0:500], embw1T[:], seqT5[:, th * 500:(th + 1) * 500], start=True, stop=True)
                nc.vector.tensor_scalar(e1[:, th * 500:(th + 1) * 500], pe_[:, 0:500],
                                        embb1[:], None, ALU.add)
            e1r = sb((128, SP), TR, tag="hR", pool=app)
            nc.vector.tensor_scalar_max(e1r[:], e1[:].bitcast(TS), 0.0)
            xemb = sb((128, EC * SP), TR, tag="resid", pool=app)
            for co in range(EC):
                for th in range(2):
                    px = pt((128, 512))
                    MM(px[:, 0:500], embw2T[:, co * 128:(co + 1) * 128],
                       e1r[:, th * 500:th * 500 + 500], start=True, stop=True)
                    nc.vector.tensor_scalar(xemb[:, co * SP + th * 500: co * SP + (th + 1) * 500],
                                            px[:, 0:500], embb2[:, co:co + 1], None, ALU.add)

            def ln_T(xin_r, g_fn, b_fn, extra_fn=None):
                # xin_r: f32r tile [128, EC*SP]
                xr = xin_r
                x2 = sb((128, EC * SP), TR, tag="att", pool=app)
                for co in range(EC):
                    nc.vector.tensor_mul(x2[:, co * SP:co * SP + 1000],
                                         xin_r[:, co * SP:co * SP + 1000].bitcast(TS),
                                         xin_r[:, co * SP:co * SP + 1000].bitcast(TS))
                pstS = pt((1, 1024), tag="b")
                for co in range(EC):
                    for th in range(2):
                        MMs(pstS[0:1, th * 512:th * 512 + 500], ones128r[:],
                           xr[:, co * SP + th * 500: co * SP + (th + 1) * 500],
                           start=(co == 0), stop=(co == EC - 1))
                sums = sb((1, 1024), tag="row1k")
                nc.vector.tensor_scalar_mul(sums[:], pstS[:], 1.0 / 256.0)
                meanr = sb((1, 1024), TR, tag="row1k")
                copy(meanr[:], sums[:])
                pstQ = pt((1, 1024), tag="b")
                for co in range(EC):
                    for th in range(2):
                        MMs(pstQ[0:1, th * 512:th * 512 + 500], ones128r[:],
                           x2[:, co * SP + th * 500: co * SP + (th + 1) * 500],
                           start=(co == 0), stop=(co == EC - 1))
                sqs = sb((1, 1024), tag="row1k")
                nc.vector.tensor_scalar_mul(sqs[:], pstQ[:], 1.0 / 256.0)
                m2_ = sb((1, 1024), tag="row1k")
                nc.vector.tensor_mul(m2_[:], meanr[:].bitcast(TS), meanr[:].bitcast(TS))
                var = sb((1, 1024), tag="row1k")
                nc.vector.tensor_sub(var[:], sqs[:], m2_[:])
                lnv_ = sb((1, 1024), tag="row1k")
                nc.scalar.activation(lnv_[:], var[:], AF.Ln, bias=epsb[0:1, :])
                rstd = sb((1, 1024), TR, tag="row1k")
                nc.scalar.activation(rstd[:], lnv_[:], AF.Exp, scale=-0.5)
                out = sb((128, EC * SP), TR, tag="x_ln", pool=app)
                for co in range(EC):
                    pmb = pt((128, 1024), tag="b")
                    for hh in range(2):
                        MMs(pmb[:, hh * 512:(hh + 1) * 512], o1x128r[:],
                           meanr[:, hh * 512:(hh + 1) * 512], start=True, stop=True)
                    xc = sb((128, 1024))
                    for th in range(2):
                        nc.vector.tensor_sub(xc[:, th * 512:th * 512 + 500],
                                             xr[:, co * SP + th * 500:co * SP + (th + 1) * 500].bitcast(TS),
                                             pmb[:, th * 512:th * 512 + 500])
                    prb2 = pt((128, 1024), tag="b")
                    for hh in range(2):
                        MMs(prb2[:, hh * 512:(hh + 1) * 512], o1x128r[:],
                           rstd[:, hh * 512:(hh + 1) * 512], start=True, stop=True)
                    nc.vector.tensor_mul(xc[:], xc[:], prb2[:])
                    if extra_fn is None:
                        for th in range(2):
                            nc.vector.tensor_scalar(out[:, co * SP + th * 500: co * SP + (th + 1) * 500],
                                                    xc[:, th * 512:th * 512 + 500],
                                                    g_fn(co), b_fn(co), ALU.mult, ALU.add)
                    else:
                        tmpe = sb((128, 1024))
                        for th in range(2):
                            nc.vector.tensor_scalar(tmpe[:, th * 512:th * 512 + 500],
                                                    xc[:, th * 512:th * 512 + 500],
                                                    g_fn(co), b_fn(co), ALU.mult, ALU.add)
                        pe2 = tmpe[:].rearrange("p (th s) -> p th s", th=2)[:, :, 0:500]
                        nc.vector.tensor_add(
                            out[:, co * SP: co * SP + 1000].rearrange("p (th s) -> p th s", s=500),
                            pe2,
                            extra_fn(co).rearrange("p (th s) -> p th s", s=500))
                    # zero pads
                    copy(out[:, co * SP + 1000: co * SP + 1024], zeros128[:, 0:24])
                return out

            for co in range(EC):
                copy(xemb[:, co * SP + 1000: co * SP + 1024], zeros128[:, 0:24])
            x = ln_T(xemb,
                     lambda co: embln[:, co:co + 1], lambda co: embln[:, EC + co:EC + co + 1],
                     extra_fn=lambda co: posT[:, co * SP: co * SP + 1000])

            # ================= layers =================
            for l in range(L):
                wq = wp.tile([128, EC * E], TR, tag="wq", name=_nm("wq"))
                nc.gpsimd.dma_start(wq[:], d["qwT"][:, l * EC * E:(l + 1) * EC * E])
                wk = wp.tile([128, EC * E], TR, tag="wk", name=_nm("wk"))
                nc.gpsimd.dma_start(wk[:], d["kwT"][:, l * EC * E:(l + 1) * EC * E])
                wv = wp.tile([128, EC * E], TR, tag="wv", name=_nm("wv"))
                nc.gpsimd.dma_start(wv[:], d["vwT"][:, l * EC * E:(l + 1) * EC * E])
                wo = wp.tile([128, EC * E], TR, tag="wo", name=_nm("wo"))
                nc.gpsimd.dma_start(wo[:], d["owT"][:, l * EC * E:(l + 1) * EC * E])
                w1 = wp.tile([128, EC * 1024], TR, tag="w1", name=_nm("w1"))
                nc.gpsimd.dma_start(w1[:], d["fw1T"][:, l * EC * 1024:(l + 1) * EC * 1024])
                w2 = wp.tile([128, HC * E], TR, tag="w2", name=_nm("w2"))
                nc.gpsimd.dma_start(w2[:], d["fw2T"][:, l * HC * E:(l + 1) * HC * E])
                vbr = wp.tile([1, E], TR, tag="vbr", name=_nm("vbr"))
                nc.gpsimd.dma_start(vbr[:], d["vbrow"][:, l * E:(l + 1) * E])
                bq = wp.tile([128, 3 * EC], TS, tag="bqkv", name=_nm("bq"))
                nc.sync.dma_start(bq[:], d["qkvb"][:, l * 3 * EC:(l + 1) * 3 * EC])
                bo = wp.tile([128, EC], TS, tag="bo", name=_nm("bo"))
                nc.sync.dma_start(bo[:], d["obias"][:, l * EC:(l + 1) * EC])
                b1 = wp.tile([128, HC], TS, tag="b1", name=_nm("b1"))
                nc.sync.dma_start(b1[:], d["fb1"][:, l * HC:(l + 1) * HC])
                b2 = wp.tile([128, EC], TS, tag="b2", name=_nm("b2"))
                nc.sync.dma_start(b2[:], d["fb2"][:, l * EC:(l + 1) * EC])
                lg = wp.tile([128, EC], TS, tag="lg", name=_nm("lg"))
                nc.sync.dma_start(lg[:], d["lng"][:, l * EC:(l + 1) * EC])
                lb = wp.tile([128, EC], TS, tag="lb", name=_nm("lb"))
                nc.sync.dma_start(lb[:], d["lnb"][:, l * EC:(l + 1) * EC])

                qTs = sb((128, EC * SP), TR, tag="qTs", pool=app)
                kT = sb((128, EC * SP), TR, tag="kT", pool=app)
                for (wt, outt, bofs, scale2) in ((wq, qTs, 0, True), (wk, kT, EC, False)):
                    for co in range(EC):
                        for th in range(2):
                            pp = pt((128, 512))
                            for ci in range(EC):
                                MM(pp[:, 0:500],
                                   wt[:, (ci * EC + co) * 128:(ci * EC + co + 1) * 128],
                                   x[:, ci * SP + th * 500: ci * SP + (th + 1) * 500],
                                   start=(ci == 0), stop=(ci == EC - 1))
                            sl = outt[:, co * SP + th * 500: co * SP + (th + 1) * 500]
                            if scale2:
                                nc.vector.tensor_scalar(sl, pp[:, 0:500],
                                                        bq[:, bofs + co: bofs + co + 1],
                                                        tsB[:, l:l + 1], ALU.add, ALU.mult)
                            else:
                                nc.vector.tensor_scalar(sl, pp[:, 0:500],
                                                        bq[:, bofs + co: bofs + co + 1], None, ALU.add)
                    for co in range(EC):
                        copy(outt[:, co * SP + 1000: co * SP + 1024], zeros128[:, 0:24])

                # V token-major: Vtm [128, tc*(H*33)] (init: zeros + masked ones cols)
                Vtm = sb((128, 8 * 264), TR, tag="Vtm", pool=app)
                vslice = Vtm[:].rearrange("p (tc h c) -> p tc h c", tc=8, h=H)
                for tcb in range(8):
                    copy(vslice[:, tcb, :, 32:33],
                         vmask8[:, tcb * 8:(tcb + 1) * 8].rearrange("p (h o) -> p h o", o=1))
                nc.vector.tensor_copy(
                    vslice[96:128, 7, :, 0:32],
                    zeros128[0:32, 0:256].rearrange("p (h dd) -> p h dd", h=H))
                for tcb in range(8):
                    pv2 = pt((128, 512))
                    for ci in range(EC):
                        MM(pv2[:, 0:256],
                           x[:, ci * SP + tcb * 128: ci * SP + (tcb + 1) * 128],
                           wv[:, ci * E:(ci + 1) * E],
                           start=(ci == 0), stop=False)
                    MMs(pv2[:, 0:256], o1x128r[:], vbr[:], start=False, stop=True)
                    nrows = 128 if tcb < 7 else 104
                    nc.vector.tensor_copy(
                        vslice[0:nrows, tcb, :, 0:32],
                        pv2[0:nrows, 0:256].rearrange("p (h dd) -> p h dd", h=H))

                att = sb((128, EC * SP), TR, tag="att", pool=app)
                for hh in range(H):
                    co_h, r0 = hh // 4, (hh % 4) * 32
                    uai = sb((33, 1024), tag="uai", pool=app)
                    expsT = sb((128, 4096), TR, tag="expsT", pool=app)
                    ev = expsT[:].rearrange("p (t s) -> p t s", s=512)
                    for qq in range(4):
                        half = (qq % 2) * 256
                        for tch in range(2):
                            psc = pt((128, 1024), tag="sc")
                            for tcb4 in range(4):
                                tcb = tch * 4 + tcb4
                                MM(psc[:, tcb4 * 256: (tcb4 + 1) * 256],
                                   kT[r0:r0 + 32, co_h * SP + tcb * 128: co_h * SP + (tcb + 1) * 128],
                                   qTs[r0:r0 + 32, co_h * SP + qq * 256: co_h * SP + (qq + 1) * 256],
                                   start=True, stop=True, tile_position=(r0, 0))
                            nc.scalar.activation(ev[:, tch * 4:(tch + 1) * 4, half:half + 256],
                                                 psc[:].rearrange("p (t s) -> p t s", s=256),
                                                 AF.Exp)
                        if qq % 2 == 1:
                            pav = pt((33, 512), tag="pj")
                            for tcb in range(8):
                                MM(pav[:, 0:512],
                                   Vtm[:, tcb * 264 + hh * 33: tcb * 264 + (hh + 1) * 33],
                                   expsT[:, tcb * 512: (tcb + 1) * 512],
                                   start=(tcb == 0), stop=(tcb == 7))
                            copy(uai[:, (qq // 2) * 512:(qq // 2) * 512 + 512], pav[:])
                    rln = sb((1, 1024), tag="row1k")
                    nc.scalar.activation(rln[:], uai[32:33, :], AF.Ln)
                    rr = sb((1, 1024), TR, tag="row1k")
                    nc.scalar.activation(rr[:], rln[:], AF.Exp, scale=-1.0)
                    prrB = pt((32, 1024), tag="b")
                    for hh2 in range(2):
                        MMs(prrB[:, hh2 * 512:(hh2 + 1) * 512], o1x32r[:],
                           rr[:, hh2 * 512:(hh2 + 1) * 512], start=True, stop=True)
                    nc.vector.tensor_mul(
                        att[r0:r0 + 32, co_h * SP: co_h * SP + 1024],
                        uai[0:32, :], prrB[:])

                # O proj + residual
                resid = sb((128, EC * SP), TR, tag="resid", pool=app)
                for co in range(EC):
                    for th in range(2):
                        po = pt((128, 512))
                        for ci in range(EC):
                            MM(po[:, 0:500],
                               wo[:, (ci * EC + co) * 128:(ci * EC + co + 1) * 128],
                               att[:, ci * SP + th * 500: ci * SP + (th + 1) * 500],
                               start=(ci == 0), stop=(ci == EC - 1))
                        tbo = sb((128, 512))
                        nc.vector.tensor_scalar(tbo[:, 0:500], po[:, 0:500], bo[:, co:co + 1], None, ALU.add)
                        sl = resid[:, co * SP + th * 500: co * SP + (th + 1) * 500]
                        nc.vector.tensor_add(sl, tbo[:, 0:500],
                                             x[:, co * SP + th * 500: co * SP + (th + 1) * 500].bitcast(TS))
                    copy(resid[:, co * SP + 1000: co * SP + 1024], zeros128[:, 0:24])
                x = ln_T(resid,
                         lambda co, lg=lg: lg[:, co:co + 1], lambda co, lb=lb: lb[:, co:co + 1])

                # FFN
                resid2 = sb((128, EC * SP), TR, tag="resid", pool=app)
                for th in range(2):
                    hR = sb((128, HC * 512), TR, tag="hR", pool=app)
                    for hc in range(HC):
                        pf_ = pt((128, 512))
                        for ci in range(EC):
                            MM(pf_[:, 0:500],
                               w1[:, (ci * HC + hc) * 128:(ci * HC + hc + 1) * 128],
                               x[:, ci * SP + th * 500: ci * SP + (th + 1) * 500],
                               start=(ci == 0), stop=(ci == EC - 1))
                        nc.scalar.activation(hR[:, hc * 512: hc * 512 + 500],
                                             pf_[:, 0:500], AF.Gelu, bias=b1[:, hc:hc + 1])
                    for co in range(EC):
                        p2_ = pt((128, 512))
                        for hc in range(HC):
                            MM(p2_[:, 0:500],
                               w2[:, (hc * EC + co) * 128:(hc * EC + co + 1) * 128],
                               hR[:, hc * 512: hc * 512 + 500],
                               start=(hc == 0), stop=(hc == HC - 1))
                        tb2 = sb((128, 512))
                        nc.vector.tensor_scalar(tb2[:, 0:500], p2_[:, 0:500], b2[:, co:co + 1], None, ALU.add)
                        sl = resid2[:, co * SP + th * 500: co * SP + (th + 1) * 500]
                        nc.vector.tensor_add(sl, tb2[:, 0:500],
                                             x[:, co * SP + th * 500: co * SP + (th + 1) * 500].bitcast(TS))
                for co in range(EC):
                    copy(resid2[:, co * SP + 1000: co * SP + 1024], zeros128[:, 0:24])
                x = ln_T(resid2,
                         lambda co, lg=lg: lg[:, co:co + 1], lambda co, lb=lb: lb[:, co:co + 1])

            # ================= pooling + classifier =================
            pcs = pt((1, 1024), tag="b")
            for co in range(EC):
                for th in range(2):
                    MMs(pcs[:, th * 512: th * 512 + 500], ones128r[:],
                       x[:, co * SP + th * 500: co * SP + (th + 1) * 500],
                       start=(co == 0), stop=(co == EC - 1))
            pwacc = sb((1, 2), tag="pwacc")
            pwr = sb((1, 1024), TR, tag="row1k")
            for th in range(2):
                nc.scalar.activation(pwr[:, th * 512: th * 512 + 500],
                                     pcs[:, th * 512: th * 512 + 500], AF.Exp,
                                     accum_out=pwacc[:, th:th + 1])
            tot = sb((1, 1))
            nc.vector.tensor_add(tot[:], pwacc[:, 0:1], pwacc[:, 1:2])
            rtot = sb((1, 1))
            nc.vector.reciprocal(rtot[:], tot[:])
            pooled = sb((128, EC), tag="pooled")
            for co in range(EC):
                ppw = pt((128, 1024), tag="b")
                for th in range(2):
                    MMs(ppw[:, th * 512:(th + 1) * 512], o1x128r[:],
                       pwr[:, th * 512:(th + 1) * 512], start=True, stop=True)
                xw = sb((128, 1024))
                for th in range(2):
                    nc.vector.tensor_mul(xw[:, th * 512: th * 512 + 500],
                                         x[:, co * SP + th * 500: co * SP + (th + 1) * 500].bitcast(TS),
                                         ppw[:, th * 512: th * 512 + 500])
                copy(xw[:, 500:512], zeros128[:, 0:12])
                copy(xw[:, 1012:1024], zeros128[:, 0:12])
                nc.vector.tensor_reduce(pooled[:, co:co + 1], xw[:], AX.X, ALU.add)
            # scale by 1/total
            rtotr = sb((1, 1), TR)
            copy(rtotr[:], rtot[:])
            prt = pt((128, 1))
            MMs(prt[:], o1x128r[:], rtotr[:], start=True, stop=True)
            rtb = sb((128, 1))
            copy(rtb[:], prt[:])
            nc.vector.tensor_scalar(pooled[:], pooled[:], rtb[:, 0:1], None, ALU.mult)
            # LN over the 256-vector
            poor = sb((128, EC), TR, tag="poor")
            copy(poor[:], pooled[:])
            poo2 = sb((128, EC), TR, tag="poo2")
            nc.vector.tensor_mul(poo2[:], pooled[:], pooled[:])
            pcs2 = pt((EC, 2))
            MMs(pcs2[:, 0:1], poor[:], ones128r[:], start=True, stop=True)
            MMs(pcs2[:, 1:2], poo2[:], ones128r[:], start=True, stop=True)
            cs2 = sb((EC, 2), TR)
            copy(cs2[:], pcs2[:])
            pcs3 = pt((2, 1))
            MMs(pcs3[:], cs2[:], onesECr[:], start=True, stop=True)
            cs3t = sb((2, 1), TR)
            copy(cs3t[:], pcs3[:])
            pcs4 = pt((1, 2))
            MMs(pcs4[:], cs3t[:], I4r[0:2, 0:2], start=True, stop=True)
            cs3 = sb((1, 2))
            nc.vector.tensor_scalar_mul(cs3[:], pcs4[:], 1.0 / 256.0)
            cm2 = sb((1, 1))
            nc.vector.tensor_mul(cm2[:], cs3[0:1, 0:1], cs3[0:1, 0:1])
            cvar = sb((1, 1))
            nc.vector.tensor_sub(cvar[:], cs3[0:1, 1:2], cm2[:])
            clnv = sb((1, 1))
            nc.scalar.activation(clnv[:], cvar[:], AF.Ln, bias=epsb[0:1, :])
            crstd = sb((1, 1), TR)
            nc.scalar.activation(crstd[:], clnv[:], AF.Exp, scale=-0.5)
            cmeanr = sb((1, 1), TR)
            copy(cmeanr[:], cs3[0:1, 0:1])
            pcb = pt((128, 2))
            MMs(pcb[:, 0:1], o1x128r[:], cmeanr[:], start=True, stop=True)
            MMs(pcb[:, 1:2], o1x128r[:], crstd[:], start=True, stop=True)
            yv = sb((128, EC), TR, tag="yv")
            for co in range(EC):
                t_ = sb((128, 1))
                nc.vector.tensor_sub(t_[:], pooled[:, co:co + 1], pcb[:, 0:1])
                nc.vector.tensor_scalar(t_[:], t_[:], pcb[:, 1:2], None, ALU.mult)
                nc.vector.tensor_scalar(yv[:, co:co + 1], t_[:], clng[:, co:co + 1],
                                        clnb[:, co:co + 1], ALU.mult, ALU.add)
            pz = pt((128, 1))
            for co in range(EC):
                MMs(pz[:], cw1T[:, co * 128:(co + 1) * 128], yv[:, co:co + 1],
                   start=(co == 0), stop=(co == EC - 1))
            zv = sb((128, 1), TR)
            nc.vector.tensor_scalar(zv[:], pz[:], cb1[:], None, ALU.add)
            nc.vector.tensor_scalar_max(zv[:].bitcast(TS), zv[:].bitcast(TS), 0.0)
            zv2 = sb((128, 1), TR)
            copy(zv2[:], zv[:].bitcast(TS))
            pout = pt((NCls, 1))
            MMs(pout[:], cw2T[:], zv2[:], start=True, stop=True)
            outv = sb((NCls, 1))
            nc.vector.tensor_scalar(outv[:], pout[:], cb2[:], None, ALU.add)
            nc.sync.dma_start(out_d, outv[:])

    nc.compile()
    return nc


_NC_CACHE = {}


def _host_inputs(inputs):
    I = {k: np.asarray(v, F32) for k, v in inputs.items()}
    h = {}
    h["embw1T"] = np.ascontiguousarray(I["emb_w1"].T)                       # [5,128]
    h["embb1"] = I["emb_b1"].reshape(128, 1)

    def wT(w):
        O, II = w.shape
        return np.ascontiguousarray(w.T.reshape(II // 128, 128, O).transpose(1, 0, 2)).reshape(128, -1)

    h["embw2T"] = wT(I["emb_w2"])                                           # [128, 256]
    h["embb2"] = np.ascontiguousarray(I["emb_b2"].reshape(EC, 128).T)
    h["embln"] = np.concatenate([I["emb_ln_g"].reshape(EC, 128).T,
                                 I["emb_ln_b"].reshape(EC, 128).T], axis=1)
    posT = np.zeros((128, EC * SP), F32)
    pe = I["pos_enc"][:S]                                                   # [1000, 256]
    for co in range(EC):
        posT[:, co * SP: co * SP + S] = pe[:, co * 128:(co + 1) * 128].T
    h["posT"] = posT
    for nm_, key in (("qwT", "qw"), ("kwT", "kw"), ("vwT", "vw"), ("owT", "ow")):
        h[nm_] = np.concatenate([wT(I[key][l]) for l in range(L)], axis=1)
    h["qkvb"] = np.concatenate(
        [np.concatenate([I["qb"][l].reshape(EC, 128).T, I["kb"][l].reshape(EC, 128).T,
                         I["vb"][l].reshape(EC, 128).T], axis=1) for l in range(L)], axis=1)
    h["obias"] = np.concatenate([I["ob"][l].reshape(EC, 128).T for l in range(L)], axis=1)
    h["vbrow"] = I["vb"].reshape(1, L * E)
    h["fw1T"] = np.concatenate([wT(I["f_w1"][l]) for l in range(L)], axis=1)
    h["fw2T"] = np.concatenate([wT(I["f_w2"][l]) for l in range(L)], axis=1)
    h["fb1"] = np.concatenate([I["f_b1"][l].reshape(HC, 128).T for l in range(L)], axis=1)
    h["fb2"] = np.concatenate([I["f_b2"][l].reshape(EC, 128).T for l in range(L)], axis=1)
    h["lng"] = np.concatenate([I["ln_g"][l].reshape(EC, 128).T for l in range(L)], axis=1)
    h["lnb"] = np.concatenate([I["ln_b"][l].reshape(EC, 128).T for l in range(L)], axis=1)
    h["pew1T"] = np.concatenate([np.ascontiguousarray(I["pe_w1"][l].T) for l in range(L)], axis=1)
    h["peb1"] = np.stack([I["pe_b1"][l] for l in range(L)], axis=1)
    h["pew2T"] = np.concatenate([wT(I["pe_w2"][l]) for l in range(L)], axis=1)
    h["peb2"] = np.concatenate([I["pe_b2"][l].reshape(EC, 128).T for l in range(L)], axis=1)
    h["clng"] = np.ascontiguousarray(I["c_ln_g"].reshape(EC, 128).T)
    h["clnb"] = np.ascontiguousarray(I["c_ln_b"].reshape(EC, 128).T)
    h["cw1T"] = wT(I["c_w1"])
    h["cb1"] = I["c_b1"].reshape(128, 1)
    h["cw2T"] = np.ascontiguousarray(I["c_w2"].T)                           # [128, 2]
    h["cb2"] = I["c_b2"].reshape(NCls, 1)
    h["ph_law"] = I["ph_law"].reshape(1, 2); h["ph_lab"] = I["ph_lab"].reshape(1, 1)
    h["ph_fw"] = I["ph_fw"].reshape(1, 1); h["ph_db"] = I["ph_db"].reshape(1, 1)
    tc_ = (np.arange(8)[None, :] * 128 + np.arange(128)[:, None]).astype(F32)
    h["tconst"] = tc_
    h["padneg"] = np.where(tc_ < S, F32(0), F32(-3e38)).astype(F32)
    h["vmask"] = (tc_ < S).astype(F32)
    h["iota50"] = np.broadcast_to(np.arange(50, dtype=F32), (128, 50)).copy()
    h["I50"] = np.eye(50, dtype=F32); h["maskD50"] = np.eye(50, dtype=F32)
    h["I4"] = np.eye(4, dtype=F32); h["I128"] = np.eye(128, dtype=F32)
    h["ones128"] = np.ones((128, 1), F32); h["ones50"] = np.ones((50, 1), F32)
    h["ones1x128"] = np.ones((1, 128), F32); h["ones1x50"] = np.ones((1, 50), F32)
    h["ones1x32"] = np.ones((1, 32), F32); h["ones4"] = np.ones((4, 1), F32)
    h["onesEC"] = np.ones((EC, 1), F32)
    v0 = np.full((50, 1), 0.1414, F32); v0[::2, 0] *= -1
    h["v0"] = v0
    h["W0"] = (np.random.default_rng(1234).standard_normal((50, 4)).astype(F32) * F32(0.14))
    h["zeros128"] = np.zeros((128, 256), F32)
    h["epsb"] = np.full((128, 1), 1e-5, F32)
    vm = (tc_ < S).astype(F32)
    h["vmask8"] = np.repeat(vm, 8, axis=1)
    return h


def kernel(**inputs):
    if "nc" not in _NC_CACHE:
        _NC_CACHE["nc"] = build_nc()
    nc = _NC_CACHE["nc"]
    h = _host_inputs(inputs)
    seqs = np.asarray(inputs["sequences"], F32)
    in_maps = []
    for b in range(4):
        m = dict(h)
        seqp = np.zeros((SP, 5), F32)
        seqp[:S] = seqs[b]
        m["seqT5"] = np.ascontiguousarray(seqp.T)
        m["seqPH"] = np.ascontiguousarray(
            seqp.reshape(8, 128, 5).transpose(1, 0, 2).reshape(128, 40))
        in_maps.append(m)
    res = bass_utils.run_bass_kernel_spmd(nc, in_maps, core_ids=[0, 1, 2, 3])
    out = np.stack([res.results[b]["out"][:, 0] for b in range(4)], axis=0)
    return out.astype(np.float32)



# revision 12
# speedup vs baseline: 1.5536x; 1.5536x over previous
"""DeepTDATransformer TRN2 Bass kernel: 4-core sample-parallel, bf16 core."""
import numpy as np
import ml_dtypes
import concourse.bacc as bacc
import concourse.tile as tile
import concourse.mybir as mybir
from concourse import bass_utils

dt = mybir.dt
AF = mybir.ActivationFunctionType
ALU = mybir.AluOpType
AX = mybir.AxisListType
F32 = np.float32
B16 = ml_dtypes.bfloat16
TS = dt.float32
TR = dt.float32r
TB = dt.bfloat16

S, SP, E, H, DH, L, NCls = 1000, 1024, 256, 8, 32, 6, 2
EC = 2   # e chunks
HC = 8   # ffn hidden chunks

_uid = [0]


def _nm(p="i"):
    _uid[0] += 1
    return f"{p}{_uid[0]}"


def build_nc():
    nc = bacc.Bacc("TRN2", target_bir_lowering=False, debug=False, num_devices=4)
    d = {}

    def din(name, shape, dty=dt.float32):
        d[name] = nc.dram_tensor(name, list(shape), dty, kind="ExternalInput").ap()

    # bf16 transformer tensors
    din("seqT5", (5, SP), TB)
    din("embw1T", (5, 128), TB); din("embw2T", (128, EC * 128), TB)
    din("posT", (128, EC * SP), TB)
    din("qwT", (128, L * EC * E), TB); din("kwT", (128, L * EC * E), TB)
    din("vwT", (128, L * EC * E), TB); din("owT", (128, L * EC * E), TB)
    din("vbrow", (1, L * E), TB)
    din("fw1T", (128, L * EC * 1024), TB); din("fw2T", (128, L * HC * E), TB)
    din("pwrT", (1, 128), TB)  # bf16 ones row
    din("ones128b", (128, 1), TB); din("ones1x128b", (1, 128), TB)
    din("ones1x32b", (1, 32), TB)
    # fp32 biases / scales (per-partition scalars)
    din("embb1", (128, 1)); din("embb2", (128, EC))
    din("embln", (128, 2 * EC))
    din("qkvb", (128, L * 3 * EC)); din("obias", (128, L * EC))
    din("fb1", (128, L * HC)); din("fb2", (128, L * EC))
    din("lng", (128, L * EC)); din("lnb", (128, L * EC))
    # PE-net (ts) fp32
    din("pew1T", (6, L * 128)); din("peb1", (128, L))
    din("pew2T", (128, L * E)); din("peb2", (128, L * EC))
    # classifier fp32
    din("clng", (128, EC)); din("clnb", (128, EC))
    din("cw1T", (128, EC * 128)); din("cb1", (128, 1))
    din("cw2T", (128, NCls)); din("cb2", (NCls, 1))
    # PH fp32
    din("seqPH", (128, 40))
    din("ph_law", (1, 2)); din("ph_lab", (1, 1)); din("ph_fw", (1, 1)); din("ph_db", (1, 1))
    din("tconst", (128, 8)); din("padneg", (128, 8)); din("vmask", (128, 8))
    din("iota50", (128, 50)); din("I50", (50, 50)); din("maskD50", (50, 50)); din("I4", (4, 4))
    din("I128", (128, 128)); din("ones128", (128, 1)); din("ones50", (50, 1))
    din("ones1x128", (1, 128)); din("ones1x50", (1, 50))
    din("ones4", (4, 1)); din("onesEC", (EC, 1))
    din("v0", (50, 1)); din("W0", (50, 4)); din("zeros128", (128, 256))
    din("epsb", (128, 1))
    din("vmask8", (128, 64))
    out_d = nc.dram_tensor("out", [NCls, 1], dt.float32, kind="ExternalOutput").ap()

    with tile.TileContext(nc) as tc:
        with (
            tc.tile_pool(name="const", bufs=1) as cp,
            tc.tile_pool(name="wp", bufs=2) as wp,
            tc.tile_pool(name="ap_", bufs=1) as app,
            tc.tile_pool(name="sm", bufs=2) as sm,
            tc.tile_pool(name="sm4", bufs=2) as sm4,
            tc.tile_pool(name="rp", bufs=4) as rp,
            tc.tile_pool(name="psA", bufs=2, space="PSUM") as psA,
            tc.tile_pool(name="psB", bufs=2, space="PSUM") as psB,
            tc.tile_pool(name="psC", bufs=2, space="PSUM") as psC,
        ):
            def c32(name, shape, nm=None, dty=TS):
                t = cp.tile(list(shape), dty, tag=nm or name, name=nm or name)
                nc.sync.dma_start(t[:], d[name])
                return t

            def c32r(name, shape, nm=None):
                t = cp.tile(list(shape), TR, tag=(nm or name) + "r", name=(nm or name) + "r")
                nc.gpsimd.dma_start(t[:], d[name])
                return t

            def cb16(name, shape, nm=None):
                t = cp.tile(list(shape), TB, tag=(nm or name) + "b", name=(nm or name) + "b")
                nc.sync.dma_start(t[:], d[name])
                return t

            # bf16 consts
            seqT5 = cb16("seqT5", (5, SP))
            embw1T = cb16("embw1T", (5, 128)); embw2T = cb16("embw2T", (128, EC * 128))
            posT = cb16("posT", (128, EC * SP))
            o1x128b = cb16("ones1x128b", (1, 128)); ones128b = cb16("ones128b", (128, 1))
            o1x32b = cb16("ones1x32b", (1, 32))
            # fp32 consts
            embb1 = c32("embb1", (128, 1)); embb2 = c32("embb2", (128, EC))
            embln = c32("embln", (128, 2 * EC))
            seqPH = c32("seqPH", (128, 40))
            tconst = c32("tconst", (128, 8)); padneg = c32("padneg", (128, 8))
            iota50 = c32("iota50", (128, 50))
            I50r = c32r("I50", (50, 50)); maskD50 = c32("maskD50", (50, 50))
            I4 = c32("I4", (4, 4)); I4r = c32r("I4", (4, 4), "I4c"); I128r = c32r("I128", (128, 128))
            ones128r = c32r("ones128", (128, 1)); ones50r = c32r("ones50", (50, 1))
            o1x128r = c32r("ones1x128", (1, 128)); o1x50r = c32r("ones1x50", (1, 50))
            ones4r = c32r("ones4", (4, 1))
            onesECr = c32r("onesEC", (EC, 1))
            v0 = c32r("v0", (50, 1)); W0r = c32r("W0", (50, 4))
            zeros128 = c32("zeros128", (128, 256))
            vmask8 = c32("vmask8", (128, 64))
            epsb = c32("epsb", (128, 1))
            pew1T = c32r("pew1T", (6, L * 128)); peb1 = c32("peb1", (128, L))
            pew2T = c32r("pew2T", (128, L * E)); peb2 = c32("peb2", (128, L * EC))
            clng = c32("clng", (128, EC)); clnb = c32("clnb", (128, EC))
            cw1T = c32r("cw1T", (128, EC * 128)); cb1 = c32("cb1", (128, 1))
            cw2T = c32r("cw2T", (128, NCls)); cb2 = c32("cb2", (NCls, 1))
            law = c32r("ph_law", (1, 2)); lab = c32r("ph_lab", (1, 1))
            phfw = c32r("ph_fw", (1, 1)); phdb = c32r("ph_db", (1, 1))

            def pt(shape, tag="pj"):
                pool = {"sc": psA, "pj": psB, "pav": psC}[tag]
                return pool.tile(list(shape), TS, tag=tag, name=_nm("p"))

            def sb(shape, dtype=TS, pool=sm, tag=None, bufs=None):
                if tag is None:
                    fbytes = int(np.prod(shape[1:])) * mybir.dt.size(dtype)
                    if fbytes >= 2048:
                        return sm4.tile(list(shape), dtype, tag=f"g{fbytes}", name=_nm("s"))
                    tag = _nm("t")
                elif tag in ("row1k",):
                    return rp.tile(list(shape), dtype, tag=tag, name=_nm("s"))
                return pool.tile(list(shape), dtype, tag=tag, name=_nm("s"), bufs=bufs)

            def copy(dst, src):
                nc.vector.tensor_copy(dst, src)

            MM = nc.tensor.matmul

            def MMs(out, lhsT, rhs, **kw):
                l2 = lhsT.bitcast(TS) if lhsT.dtype == TR else lhsT
                r2 = rhs.bitcast(TS) if rhs.dtype == TR else rhs
                return MM(out, l2, r2, **kw)

            # ================= PH (fp32, unchanged math) =================
            mfeat = sb((128, 8))
            nc.vector.tensor_reduce(mfeat[:], seqPH[:].rearrange("p (c f) -> p c f", f=5),
                                    AX.X, ALU.add)
            nc.vector.tensor_scalar_mul(mfeat[:], mfeat[:], 0.2)
            p1 = pt((128, 8), tag="pav")
            MMs(p1[:, 0:2], o1x128r[:], law[:], start=True, stop=True)
            MMs(p1[:, 2:3], o1x128r[:], lab[:], start=True, stop=True)
            lawB = sb((128, 4))
            copy(lawB[:], p1[:, 0:4])
            scs = sb((128, 8))
            nc.vector.tensor_scalar(scs[:], tconst[:], lawB[:, 0:1], None, ALU.mult)
            tmp8 = sb((128, 8))
            nc.vector.tensor_scalar(tmp8[:], mfeat[:], lawB[:, 1:2], None, ALU.mult)
            nc.vector.tensor_add(scs[:], scs[:], tmp8[:])
            nc.vector.tensor_scalar(scs[:], scs[:], lawB[:, 2:3], None, ALU.add)
            nc.vector.tensor_add(scs[:], scs[:], padneg[:])
            scr = sb((128, 8), TR)
            copy(scr[:], scs[:])
            srow = sb((1, 1024), TR, tag="row1k")
            for hf in range(2):
                p2 = pt((1, 512), tag="pav")
                for c in range(4):
                    cc = hf * 4 + c
                    MMs(p2[:, c * 128:(c + 1) * 128], scr[:, cc:cc + 1], I128r[:],
                        start=True, stop=True)
                copy(srow[:, hf * 512:(hf + 1) * 512], p2[:])
            sROW = sb((128, 1024))
            for hh in range(2):
                p3 = pt((128, 512), tag="pav")
                MMs(p3[:], o1x128r[:], srow[:, hh * 512:(hh + 1) * 512],
                    start=True, stop=True)
                copy(sROW[:, hh * 512:(hh + 1) * 512], p3[:])
            rank = sb((128, 8))
            scratch = sb((128, 1024))
            for c in range(8):
                nc.vector.tensor_scalar(scratch[:], sROW[:], scs[:, c:c + 1], 0.0,
                                        ALU.is_gt, ALU.add, accum_out=rank[:, c:c + 1])
            ptsr = sb((128, 16), TR)
            pv = ptsr[:].rearrange("p (c two) -> p c two", two=2)
            copy(pv[:, :, 0:1], tconst[:].rearrange("p (c o) -> p c o", o=1))
            copy(pv[:, :, 1:2], mfeat[:].rearrange("p (c o) -> p c o", o=1))
            Gc = sb((128, 400), TR, tag="Gc", pool=app)
            for c in range(8):
                nc.vector.tensor_scalar(Gc[:, c * 50:(c + 1) * 50], iota50[:],
                                        rank[:, c:c + 1], None, ALU.is_equal)
            plm = pt((50, 2), tag="pav")
            plmT = pt((2, 50), tag="pav")
            for c in range(8):
                MMs(plm[:], Gc[:, c * 50:(c + 1) * 50], ptsr[:, c * 2:(c + 1) * 2],
                   start=(c == 0), stop=(c == 7))
            for c in range(8):
                MMs(plmT[:], ptsr[:, c * 2:(c + 1) * 2], Gc[:, c * 50:(c + 1) * 50],
                   start=(c == 0), stop=(c == 7))
            lmT = sb((2, 50), TR)
            copy(lmT[:], plmT[:])
            pg = pt((50, 50), tag="pav")
            MMs(pg[:], lmT[:], lmT[:], start=True, stop=True)
            gram = sb((50, 50))
            copy(gram[:], pg[:])
            sqd = sb((50, 50))
            nc.vector.tensor_mul(sqd[:], gram[:], maskD50[:])
            sq = sb((50, 1))
            nc.vector.tensor_reduce(sq[:], sqd[:], AX.X, ALU.add)
            t1 = sb((50, 50))
            nc.vector.tensor_scalar(t1[:], gram[:], -2.0, sq[:], ALU.mult, ALU.add)
            sqr = sb((50, 1), TR)
            copy(sqr[:], sq[:])
            p4 = pt((1, 50), tag="pav")
            MMs(p4[:], sqr[:], I50r[:], start=True, stop=True)
            sqrow = sb((1, 50), TR)
            copy(sqrow[:], p4[:])
            p5 = pt((50, 50), tag="pav")
            MMs(p5[:], o1x50r[:], sqrow[:], start=True, stop=True)
            d2 = sb((50, 50))
            nc.vector.tensor_add(d2[:], t1[:], p5[:])
            nc.vector.tensor_scalar_max(d2[:], d2[:], 1e-30)
            lnd = sb((50, 50))
            nc.scalar.activation(lnd[:], d2[:], AF.Ln)
            distm = sb((50, 50))
            nc.scalar.activation(distm[:], lnd[:], AF.Exp, scale=0.5)
            p6 = pt((50, 2), tag="pav")
            MMs(p6[:, 0:1], o1x50r[:], phfw[:], start=True, stop=True)
            MMs(p6[:, 1:2], o1x50r[:], phdb[:], start=True, stop=True)
            fwdb = sb((50, 2))
            copy(fwdb[:], p6[:])
            nfw = sb((50, 2))
            nc.scalar.activation(nfw[:, 0:1], fwdb[:, 0:1], AF.Abs)
            nc.vector.tensor_scalar_mul(nfw[:, 1:2], fwdb[:, 1:2], -1.0)
            dists = sb((50, 50))
            nc.vector.tensor_scalar(dists[:], distm[:], nfw[:, 0:1], None, ALU.mult)
            Km = sb((50, 50))
            nc.scalar.activation(Km[:], dists[:], AF.Exp, scale=-1.0, bias=nfw[:, 1:2])
            s_r = sb((50, 1))
            nc.vector.tensor_reduce(s_r[:], Km[:], AX.X, ALU.add)
            Bm = sb((50, 50))
            nc.vector.tensor_scalar(Bm[:], maskD50[:], s_r[:], None, ALU.mult)
            nc.vector.tensor_sub(Bm[:], Bm[:], Km[:])
            nc.vector.tensor_scalar_mul(Bm[:], Bm[:], -1.0)
            D40 = sb((50, 50))
            nc.vector.tensor_scalar_mul(D40[:], maskD50[:], 40.0)
            nc.vector.tensor_add(Bm[:], Bm[:], D40[:])
            nc.vector.tensor_scalar_add(Bm[:], Bm[:], -0.8)
            Br = sb((50, 50), TR)
            copy(Br[:], Bm[:])

            def vec_norm(vr):
                pn = pt((1, 1), tag="pav")
                MMs(pn[:], vr[:], vr[:], start=True, stop=True)
                lnv = sb((1, 1))
                nc.scalar.activation(lnv[:], pn[:], AF.Ln)
                rs = sb((1, 1), TR)
                nc.scalar.activation(rs[:], lnv[:], AF.Exp, scale=-0.5)
                prb = pt((50, 1), tag="pav")
                MMs(prb[:], o1x50r[:], rs[:], start=True, stop=True)
                vn = sb((50, 1), TR, tag="vpow")
                nc.vector.tensor_mul(vn[:].bitcast(TS), vr[:].bitcast(TS), prb[:])
                vn2 = sb((50, 1), TR, tag="vpow")
                copy(vn2[:], vn[:].bitcast(TS))
                return vn2

            v = v0
            for it in range(12):
                pv_ = pt((50, 1), tag="pav")
                MMs(pv_[:], Br[:], v[:], start=True, stop=True)
                v = sb((50, 1), TR, tag="vpow")
                nc.vector.tensor_scalar_mul(v[:], pv_[:], 0.125)
                if it % 4 == 3:
                    v = vec_norm(v)
            v = vec_norm(v)
            pbv = pt((50, 1), tag="pav")
            MMs(pbv[:], Br[:], v[:], start=True, stop=True)
            vbvf = sb((50, 1))
            nc.vector.tensor_mul(vbvf[:], v[:].bitcast(TS), pbv[:])
            vbv = sb((50, 1), TR)
            copy(vbv[:], vbvf[:])
            pmu = pt((1, 1), tag="pav")
            MMs(pmu[:], vbv[:], ones50r[:], start=True, stop=True)
            mu1 = sb((1, 1))
            copy(mu1[:], pmu[:])
            pvr = pt((1, 50), tag="pav")
            MMs(pvr[:], v[:], I50r[:], start=True, stop=True)
            vRow = sb((1, 50), TR)
            copy(vRow[:], pvr[:])

            def ns_orth(W, nstep):
                pg_ = pt((4, 4), tag="pav")
                MMs(pg_[:], W[:], W[:], start=True, stop=True)
                gd = sb((4, 4))
                nc.vector.tensor_mul(gd[:], pg_[:], I4[:])
                gdr = sb((4, 1))
                nc.vector.tensor_reduce(gdr[:], gd[:], AX.X, ALU.add)
                gdr2 = sb((4, 1), TR)
                copy(gdr2[:], gdr[:])
                ptr = pt((1, 1), tag="pav")
                MMs(ptr[:], gdr2[:], ones4r[:], start=True, stop=True)
                lnt = sb((1, 1))
                nc.scalar.activation(lnt[:], ptr[:], AF.Ln, scale=0.25)
                rst = sb((1, 1), TR)
                nc.scalar.activation(rst[:], lnt[:], AF.Exp, scale=-0.5)
                prb = pt((50, 1), tag="pav")
                MMs(prb[:], o1x50r[:], rst[:], start=True, stop=True)
                Wn = sb((50, 4), TR, tag="Wsub")
                nc.vector.tensor_scalar(Wn[:], W[:].bitcast(TS), prb[:], None, ALU.mult)
                W = Wn
                for _ in range(nstep):
                    pg2 = pt((4, 4), tag="pav")
                    MMs(pg2[:], W[:], W[:], start=True, stop=True)
                    i4h = sb((4, 4))
                    nc.vector.tensor_scalar_mul(i4h[:], I4[:], 1.5)
                    corrf = sb((4, 4))
                    nc.vector.tensor_scalar(corrf[:], pg2[:], -0.5, None, ALU.mult)
                    corr = sb((4, 4), TR)
                    nc.vector.tensor_add(corr[:], corrf[:], i4h[:])
                    pwt = pt((4, 50), tag="pav")
                    MMs(pwt[:], W[:], I50r[:], start=True, stop=True)
                    WT = sb((4, 50), TR)
                    copy(WT[:], pwt[:])
                    pw2 = pt((50, 4), tag="pav")
                    MMs(pw2[:], WT[:], corr[:], start=True, stop=True)
                    W = sb((50, 4), TR, tag="Wsub")
                    copy(W[:], pw2[:])
                return W

            W = W0r
            for it in range(14):
                pw_ = pt((50, 4), tag="pav")
                MMs(pw_[:], Br[:], W[:], start=True, stop=True)
                Wn = sb((50, 4), TR, tag="Wsub")
                nc.vector.tensor_scalar_mul(Wn[:], pw_[:], 0.125)
                W = Wn
                pc_ = pt((1, 4), tag="pav")
                MMs(pc_[:], v[:], W[:], start=True, stop=True)
                cvw = sb((1, 4), TR)
                copy(cvw[:], pc_[:])
                pcor = pt((50, 4), tag="pav")
                MMs(pcor[:], vRow[:], cvw[:], start=True, stop=True)
                Wn = sb((50, 4), TR, tag="Wsub")
                nc.vector.tensor_sub(Wn[:].bitcast(TS), W[:].bitcast(TS), pcor[:])
                W2_ = sb((50, 4), TR, tag="Wsub")
                copy(W2_[:], Wn[:].bitcast(TS))
                W = W2_
                if it % 6 == 5:
                    W = ns_orth(W, 3)
            W = ns_orth(W, 6)
            pbw = pt((50, 4), tag="pav")
            MMs(pbw[:], Br[:], W[:], start=True, stop=True)
            BW = sb((50, 4), TR)
            copy(BW[:], pbw[:])
            ph4 = pt((4, 4), tag="pav")
            MMs(ph4[:], W[:], BW[:], start=True, stop=True)
            H4 = sb((4, 4))
            copy(H4[:], ph4[:])
            h4d = sb((4, 4)); h4f = sb((4, 4))
            nc.vector.tensor_mul(h4d[:], H4[:], I4[:])
            nc.vector.tensor_mul(h4f[:], H4[:], H4[:])
            rd = sb((4, 1)); rf = sb((4, 1))
            nc.vector.tensor_reduce(rd[:], h4d[:], AX.X, ALU.add)
            nc.vector.tensor_reduce(rf[:], h4f[:], AX.X, ALU.add)
            rdr = sb((4, 2), TR)
            copy(rdr[:, 0:1], rd[:]); copy(rdr[:, 1:2], rf[:])
            pst_ = pt((2, 1), tag="pav")
            MMs(pst_[:], rdr[:], ones4r[:], start=True, stop=True)
            stt2 = sb((2, 1), TR)
            copy(stt2[:], pst_[:])
            pstr = pt((1, 2), tag="pav")
            MMs(pstr[:], stt2[:], I4r[0:2, 0:2], start=True, stop=True)
            sttrow = sb((1, 2))
            copy(sttrow[:], pstr[:])
            frH0 = sttrow[0:1, 1:2]
            mean_mu = sb((1, 1))
            nc.vector.tensor_scalar_mul(mean_mu[:], sttrow[0:1, 0:1], 0.25)
            m2 = sb((1, 1))
            nc.vector.tensor_mul(m2[:], mean_mu[:], mean_mu[:])
            nc.vector.tensor_scalar_mul(m2[:], m2[:], -4.0 / 3.0)
            varq = sb((1, 1))
            nc.vector.tensor_scalar_mul(varq[:], frH0[:], 1.0 / 3.0)
            nc.vector.tensor_add(varq[:], varq[:], m2[:])
            nc.vector.tensor_scalar_max(varq[:], varq[:], 1e-30)
            lnv3 = sb((1, 1))
            nc.scalar.activation(lnv3[:], varq[:], AF.Ln)
            std_ev = sb((1, 1))
            nc.scalar.activation(std_ev[:], lnv3[:], AF.Exp, scale=0.5)
            mean_ev = sb((1, 1))
            nc.vector.tensor_scalar(mean_ev[:], mean_mu[:], -1.0, 40.0, ALU.mult, ALU.add)
            gap = sb((1, 1))
            nc.vector.tensor_scalar(gap[:], mu1[:], -1.0, 40.0, ALU.mult, ALU.add)
            pfrow = sb((1, 8))
            copy(pfrow[:], zeros128[0:1, 0:8])
            nc.vector.tensor_scalar_add(pfrow[:, 0:1], pfrow[:, 0:1], 1.0)
            nc.vector.tensor_scalar_add(pfrow[:, 3:4], pfrow[:, 3:4], 1.0 / 7.0)
            copy(pfrow[:, 2:3], gap[:])
            copy(pfrow[:, 4:5], mean_ev[:])
            copy(pfrow[:, 5:6], std_ev[:])
            pfrr = sb((1, 8), TR)
            copy(pfrr[:], pfrow[:])
            ppf = pt((8, 1), tag="pav")
            MMs(ppf[:], pfrr[:], o1x128r[:, 0:1], start=True, stop=True)
            pfr = sb((8, 1), TR)
            copy(pfr[:], ppf[:])
            pfr = pfr[0:6, :]

            # ts per layer (fp32 PE-net)
            tsB = sb((128, L), tag="tsB", pool=app)
            for l in range(L):
                ph1 = pt((128, 1), tag="pav")
                MMs(ph1[:], pew1T[:, l * 128:(l + 1) * 128], pfr[:], start=True, stop=True)
                h1f = sb((128, 1))
                nc.vector.tensor_scalar(h1f[:], ph1[:], peb1[:, l:l + 1], None, ALU.add)
                h1b = sb((128, 1), TR)
                nc.vector.tensor_scalar_max(h1b[:], h1f[:], 0.0)
                sig = sb((128, EC))
                for co in range(EC):
                    py = pt((128, 1), tag="pav")
                    MMs(py[:], pew2T[:, (l * EC + co) * 128:(l * EC + co + 1) * 128],
                       h1b[:], start=True, stop=True)
                    yb = sb((128, 1))
                    nc.vector.tensor_scalar(yb[:], py[:], peb2[:, l * EC + co:l * EC + co + 1],
                                            None, ALU.add)
                    ey = sb((128, 1))
                    nc.scalar.activation(ey[:], yb[:], AF.Exp, scale=-1.0)
                    nc.vector.tensor_scalar_add(ey[:], ey[:], 1.0)
                    nc.vector.reciprocal(sig[:, co:co + 1], ey[:])
                sigr = sb((128, EC), TR)
                copy(sigr[:], sig[:])
                pts_ = pt((EC, 1), tag="pav")
                MMs(pts_[:], sigr[:], ones128r[:], start=True, stop=True)
                tsum = sb((EC, 1), TR)
                copy(tsum[:], pts_[:])
                pt2 = pt((1, 1), tag="pav")
                MMs(pt2[:], tsum[:], onesECr[:], start=True, stop=True)
                tsv = sb((1, 1), TR)
                nc.vector.tensor_scalar_mul(tsv[:], pt2[:], float(1.0 / (256.0 * np.sqrt(32.0))))
                ptb = pt((128, 1), tag="pav")
                MMs(ptb[:], o1x128r[:], tsv[:], start=True, stop=True)
                copy(tsB[:, l:l + 1], ptb[:])

            # ================= embedding (bf16) =================
            e1r = sb((128, SP), TB, tag="e1r", pool=app)
            for th in range(2):
                pe_ = pt((128, 512), tag="pj")
                MM(pe_[:, 0:500], embw1T[:], seqT5[:, th * 500:(th + 1) * 500],
                   start=True, stop=True)
                nc.scalar.activation(e1r[:, th * 500:(th + 1) * 500], pe_[:, 0:500],
                                     AF.Relu, bias=embb1[:, 0:1])
            xemb = sb((128, EC * SP), TB, tag="resid", pool=app)
            for co in range(EC):
                for th in range(2):
                    px = pt((128, 512), tag="pj")
                    MM(px[:, 0:500], embw2T[:, co * 128:(co + 1) * 128],
                       e1r[:, th * 500:th * 500 + 500], start=True, stop=True)
                    nc.vector.tensor_scalar(xemb[:, co * SP + th * 500: co * SP + (th + 1) * 500],
                                            px[:, 0:500], embb2[:, co:co + 1], None, ALU.add)
                nc.gpsimd.tensor_copy(xemb[:, co * SP + 1000: co * SP + 1024],
                                      zeros128[:, 0:24])

            def ln_T(xr, g_fn, b_fn, extra_fn=None):
                # xr: bf16 [128, EC*SP] -> returns bf16 [128, EC*SP] LN'd
                x2 = sb((128, EC * SP), TB, tag="x2", pool=app)
                for co in range(EC):
                    nc.vector.tensor_tensor(x2[:, co * SP:co * SP + 1000],
                                            xr[:, co * SP:co * SP + 1000],
                                            xr[:, co * SP:co * SP + 1000], ALU.mult)
                pstS = pt((1, 1024), tag="sc")
                pstQ = pt((1, 1024), tag="sc")
                for th in range(2):
                    for co in range(EC):
                        MM(pstS[0:1, th * 512:th * 512 + 500], ones128b[:],
                           xr[:, co * SP + th * 500: co * SP + (th + 1) * 500],
                           start=(co == 0), stop=(co == EC - 1))
                    for co in range(EC):
                        MM(pstQ[0:1, th * 512:th * 512 + 500], ones128b[:],
                           x2[:, co * SP + th * 500: co * SP + (th + 1) * 500],
                           start=(co == 0), stop=(co == EC - 1))
                mu_b = sb((1, 1024), TB, tag="rowb", bufs=2)
                nc.vector.tensor_scalar_mul(mu_b[:], pstS[:], 1.0 / 256.0)
                e2 = sb((1, 1024), TS, tag="row1k")
                nc.vector.tensor_scalar_mul(e2[:], pstQ[:], 1.0 / 256.0)
                m2_ = sb((1, 1024), TS, tag="row1k")
                nc.gpsimd.tensor_tensor(m2_[:], mu_b[:], mu_b[:], ALU.mult)
                var = sb((1, 1024), TS, tag="row1k")
                nc.vector.tensor_tensor(var[:], e2[:], m2_[:], ALU.subtract)
                lnv_ = sb((1, 1024), TS, tag="row1k")
                nc.scalar.activation(lnv_[:], var[:], AF.Ln, bias=epsb[0:1, :])
                rstd_b = sb((1, 1024), TB, tag="rowb", bufs=2)
                nc.scalar.activation(rstd_b[:], lnv_[:], AF.Exp, scale=-0.5)
                pmb = pt((128, 1024), tag="sc")
                prb = pt((128, 1024), tag="sc")
                for hh in range(2):
                    MM(pmb[:, hh * 512:(hh + 1) * 512], o1x128b[:],
                       mu_b[:, hh * 512:(hh + 1) * 512], start=True, stop=True)
                for hh in range(2):
                    MM(prb[:, hh * 512:(hh + 1) * 512], o1x128b[:],
                       rstd_b[:, hh * 512:(hh + 1) * 512], start=True, stop=True)
                prbS = sb((128, 1024), TB, tag="prbS", bufs=2)
                nc.vector.tensor_copy(prbS[:], prb[:])
                out = sb((128, EC * SP), TB, tag="x_ln", pool=app)
                for co in range(EC):
                    for th in range(2):
                        xs = xr[:, co * SP + th * 500: co * SP + (th + 1) * 500]
                        ms = pmb[:, th * 512: th * 512 + 500]
                        rs = prbS[:, th * 512: th * 512 + 500]
                        os_ = out[:, co * SP + th * 500: co * SP + (th + 1) * 500]
                        xc = sb((128, 512), TB, tag="xc")
                        nc.vector.tensor_tensor(xc[:, 0:500], xs, ms, ALU.subtract)
                        y = sb((128, 512), TB, tag="yln")
                        nc.gpsimd.tensor_tensor(y[:, 0:500], xc[:, 0:500], rs, ALU.mult)
                        if extra_fn is None:
                            nc.vector.tensor_scalar(os_, y[:, 0:500], g_fn(co), b_fn(co),
                                                    ALU.mult, ALU.add)
                        else:
                            t2 = sb((128, 512), TB, tag="t2ln")
                            nc.vector.tensor_scalar(t2[:, 0:500], y[:, 0:500],
                                                    g_fn(co), b_fn(co), ALU.mult, ALU.add)
                            nc.gpsimd.tensor_tensor(os_, t2[:, 0:500], extra_fn(co, th),
                                                    ALU.add)
                    nc.gpsimd.tensor_copy(out[:, co * SP + 1000: co * SP + 1024],
                                          zeros128[:, 0:24])
                return out

            x = ln_T(xemb,
                     lambda co: embln[:, co:co + 1], lambda co: embln[:, EC + co:EC + co + 1],
                     extra_fn=lambda co, th: posT[:, co * SP + th * 500: co * SP + (th + 1) * 500])

            # Vtm one-time init: mask col + zero pad-key rows
            Vtm = sb((128, 8 * 264), TB, tag="Vtm", pool=app)
            vslice = Vtm[:].rearrange("p (tc h c) -> p tc h c", tc=8, h=H)
            for tcb in range(8):
                nc.gpsimd.tensor_copy(
                    vslice[:, tcb, :, 32:33],
                    vmask8[:, tcb * 8:(tcb + 1) * 8].rearrange("p (h o) -> p h o", o=1))
            nc.gpsimd.tensor_copy(
                vslice[96:128, 7, :, 0:32],
                zeros128[0:32, 0:256].rearrange("p (h dd) -> p h dd", h=H))

            # ================= layers =================
            for l in range(L):
                wq = wp.tile([128, EC * E], TB, tag="wq", name=_nm("wq"))
                nc.sync.dma_start(wq[:], d["qwT"][:, l * EC * E:(l + 1) * EC * E])
                wk = wp.tile([128, EC * E], TB, tag="wk", name=_nm("wk"))
                nc.sync.dma_start(wk[:], d["kwT"][:, l * EC * E:(l + 1) * EC * E])
                wv = wp.tile([128, EC * E], TB, tag="wv", name=_nm("wv"))
                nc.sync.dma_start(wv[:], d["vwT"][:, l * EC * E:(l + 1) * EC * E])
                wo = wp.tile([128, EC * E], TB, tag="wo", name=_nm("wo"))
                nc.sync.dma_start(wo[:], d["owT"][:, l * EC * E:(l + 1) * EC * E])
                w1 = wp.tile([128, EC * 1024], TB, tag="w1", name=_nm("w1"))
                nc.sync.dma_start(w1[:], d["fw1T"][:, l * EC * 1024:(l + 1) * EC * 1024])
                w2 = wp.tile([128, HC * E], TB, tag="w2", name=_nm("w2"))
                nc.sync.dma_start(w2[:], d["fw2T"][:, l * HC * E:(l + 1) * HC * E])
                vbr = wp.tile([1, E], TB, tag="vbr", name=_nm("vbr"))
                nc.sync.dma_start(vbr[:], d["vbrow"][:, l * E:(l + 1) * E])
                bq = wp.tile([128, 3 * EC], TS, tag="bqkv", name=_nm("bq"))
                nc.sync.dma_start(bq[:], d["qkvb"][:, l * 3 * EC:(l + 1) * 3 * EC])
                bo = wp.tile([128, EC], TS, tag="bo", name=_nm("bo"))
                nc.sync.dma_start(bo[:], d["obias"][:, l * EC:(l + 1) * EC])
                b1 = wp.tile([128, HC], TS, tag="b1", name=_nm("b1"))
                nc.sync.dma_start(b1[:], d["fb1"][:, l * HC:(l + 1) * HC])
                b2 = wp.tile([128, EC], TS, tag="b2", name=_nm("b2"))
                nc.sync.dma_start(b2[:], d["fb2"][:, l * EC:(l + 1) * EC])
                lg = wp.tile([128, EC], TS, tag="lg", name=_nm("lg"))
                nc.sync.dma_start(lg[:], d["lng"][:, l * EC:(l + 1) * EC])
                lb = wp.tile([128, EC], TS, tag="lb", name=_nm("lb"))
                nc.sync.dma_start(lb[:], d["lnb"][:, l * EC:(l + 1) * EC])

                # ---- Q, K projections (bf16) ----
                qTs = sb((128, EC * SP), TB, tag="qTs", pool=app)
                kT = sb((128, EC * SP), TB, tag="kT", pool=app)
                for (wt, outt, bofs) in ((wq, qTs, 0), (wk, kT, EC)):
                    for co in range(EC):
                        for th in range(2):
                            pp = pt((128, 512), tag="pj")
                            for ci in range(EC):
                                MM(pp[:, 0:500],
                                   wt[:, (ci * EC + co) * 128:(ci * EC + co + 1) * 128],
                                   x[:, ci * SP + th * 500: ci * SP + (th + 1) * 500],
                                   start=(ci == 0), stop=(ci == EC - 1))
                            sl = outt[:, co * SP + th * 500: co * SP + (th + 1) * 500]
                            nc.vector.tensor_scalar(sl, pp[:, 0:500],
                                                    bq[:, bofs + co: bofs + co + 1],
                                                    None, ALU.add)
                        nc.gpsimd.tensor_copy(outt[:, co * SP + 1000: co * SP + 1024],
                                              zeros128[:, 0:24])

                # ---- V (token-major, bf16) ----
                for tcb in range(8):
                    pv2 = pt((128, 512), tag="pj")
                    for ci in range(EC):
                        MM(pv2[:, 0:256],
                           x[:, ci * SP + tcb * 128: ci * SP + (tcb + 1) * 128],
                           wv[:, ci * E:(ci + 1) * E],
                           start=(ci == 0), stop=False)
                    MM(pv2[:, 0:256], o1x128b[:], vbr[:], start=False, stop=True)
                    nrows = 128 if tcb < 7 else 104
                    nc.vector.tensor_copy(
                        vslice[0:nrows, tcb, :, 0:32],
                        pv2[0:nrows, 0:256].rearrange("p (h dd) -> p h dd", h=H))

                # ---- attention per head ----
                att = sb((128, EC * SP), TB, tag="att", pool=app)
                for hh in range(H):
                    co_h, r0 = hh // 4, (hh % 4) * 32
                    expsT = sb((128, 8 * SP), TB, tag="expsT", pool=app, bufs=2)
                    for kc in range(8):
                        psc = pt((128, 1024), tag="sc")
                        for qh in range(2):
                            MM(psc[:, qh * 512:(qh + 1) * 512],
                               kT[r0:r0 + 32, co_h * SP + kc * 128: co_h * SP + (kc + 1) * 128],
                               qTs[r0:r0 + 32, co_h * SP + qh * 512: co_h * SP + (qh + 1) * 512],
                               start=True, stop=True, tile_position=(r0, 0))
                        nc.scalar.activation(expsT[:, kc * 1024:(kc + 1) * 1024], psc[:],
                                             AF.Exp, scale=tsB[:, l:l + 1])
                    pavs = []
                    for qh in range(2):
                        pav = pt((33, 512), tag="pav")
                        for kc in range(8):
                            MM(pav[:],
                               Vtm[:, kc * 264 + hh * 33: kc * 264 + (hh + 1) * 33],
                               expsT[:, kc * 1024 + qh * 512: kc * 1024 + (qh + 1) * 512],
                               start=(kc == 0), stop=(kc == 7))
                        pavs.append(pav)
                    # normalize: denom row -> reciprocal -> broadcast -> multiply
                    rdenb = sb((1, 1024), TB, tag="rdenb", bufs=2)
                    uai = sb((33, 1024), TB, tag="uai", bufs=2)
                    for qh in range(2):
                        with nc.allow_low_precision(reason="softmax denom bf16 ok"):
                            nc.vector.reciprocal(rdenb[:, qh * 512:(qh + 1) * 512],
                                                 pavs[qh][32:33, :])
                        nc.vector.tensor_copy(uai[:, qh * 512:(qh + 1) * 512], pavs[qh][:])
                    for qh in range(2):
                        prr = pt((32, 512), tag="pav")
                        MM(prr[:], o1x32b[:], rdenb[:, qh * 512:(qh + 1) * 512],
                           start=True, stop=True)
                        nc.vector.tensor_tensor(
                            att[r0:r0 + 32, co_h * SP + qh * 512: co_h * SP + (qh + 1) * 512],
                            uai[0:32, qh * 512:(qh + 1) * 512], prr[:], ALU.mult)

                # ---- O proj + residual ----
                resid = sb((128, EC * SP), TB, tag="resid", pool=app)
                for co in range(EC):
                    for th in range(2):
                        po = pt((128, 512), tag="pj")
                        for ci in range(EC):
                            MM(po[:, 0:500],
                               wo[:, (ci * EC + co) * 128:(ci * EC + co + 1) * 128],
                               att[:, ci * SP + th * 500: ci * SP + (th + 1) * 500],
                               start=(ci == 0), stop=(ci == EC - 1))
                        tbo = sb((128, 512), TB, tag="tbo")
                        nc.vector.tensor_scalar(tbo[:, 0:500], po[:, 0:500],
                                                bo[:, co:co + 1], None, ALU.add)
                        sl = resid[:, co * SP + th * 500: co * SP + (th + 1) * 500]
                        nc.gpsimd.tensor_tensor(sl, tbo[:, 0:500],
                                                x[:, co * SP + th * 500: co * SP + (th + 1) * 500],
                                                ALU.add)
                    nc.gpsimd.tensor_copy(resid[:, co * SP + 1000: co * SP + 1024],
                                          zeros128[:, 0:24])
                x = ln_T(resid,
                         lambda co, lg=lg: lg[:, co:co + 1], lambda co, lb=lb: lb[:, co:co + 1])

                # ---- FFN ----
                resid2 = sb((128, EC * SP), TB, tag="resid", pool=app)
                for th in range(2):
                    hR = sb((128, HC * 512), TB, tag="hR", pool=app, bufs=2)
                    for hc in range(HC):
                        pf_ = pt((128, 512), tag="pj")
                        for ci in range(EC):
                            MM(pf_[:, 0:500],
                               w1[:, (ci * HC + hc) * 128:(ci * HC + hc + 1) * 128],
                               x[:, ci * SP + th * 500: ci * SP + (th + 1) * 500],
                               start=(ci == 0), stop=(ci == EC - 1))
                        nc.scalar.activation(hR[:, hc * 512: hc * 512 + 500],
                                             pf_[:, 0:500], AF.Gelu, bias=b1[:, hc:hc + 1])
                    for co in range(EC):
                        p2_ = pt((128, 512), tag="pj")
                        for hc in range(HC):
                            MM(p2_[:, 0:500],
                               w2[:, (hc * EC + co) * 128:(hc * EC + co + 1) * 128],
                               hR[:, hc * 512: hc * 512 + 500],
                               start=(hc == 0), stop=(hc == HC - 1))
                        tb2 = sb((128, 512), TB, tag="tbo")
                        nc.vector.tensor_scalar(tb2[:, 0:500], p2_[:, 0:500],
                                                b2[:, co:co + 1], None, ALU.add)
                        sl = resid2[:, co * SP + th * 500: co * SP + (th + 1) * 500]
                        nc.gpsimd.tensor_tensor(sl, tb2[:, 0:500],
                                                x[:, co * SP + th * 500: co * SP + (th + 1) * 500],
                                                ALU.add)
                for co in range(EC):
                    nc.gpsimd.tensor_copy(resid2[:, co * SP + 1000: co * SP + 1024],
                                          zeros128[:, 0:24])
                x = ln_T(resid2,
                         lambda co, lg=lg: lg[:, co:co + 1], lambda co, lb=lb: lb[:, co:co + 1])

            # ================= pooling + classifier =================
            pcs = pt((1, 1024), tag="sc")
            for co in range(EC):
                for th in range(2):
                    MM(pcs[0:1, th * 512: th * 512 + 500], ones128b[:],
                       x[:, co * SP + th * 500: co * SP + (th + 1) * 500],
                       start=(co == 0), stop=(co == EC - 1))
            pwacc = sb((1, 2), tag="pwacc")
            pwr = sb((1, 1024), TB, tag="pwrb")
            for th in range(2):
                nc.scalar.activation(pwr[:, th * 512: th * 512 + 500],
                                     pcs[:, th * 512: th * 512 + 500], AF.Exp,
                                     accum_out=pwacc[:, th:th + 1])
            tot = sb((1, 1))
            nc.vector.tensor_add(tot[:], pwacc[:, 0:1], pwacc[:, 1:2])
            rtot = sb((1, 1))
            nc.vector.reciprocal(rtot[:], tot[:])
            pooled = sb((128, EC), tag="pooled")
            ppw = pt((128, 1024), tag="sc")
            for th in range(2):
                MM(ppw[:, th * 512:(th + 1) * 512], o1x128b[:],
                   pwr[:, th * 512:(th + 1) * 512], start=True, stop=True)
            for co in range(EC):
                xw = sb((128, 1024))
                for th in range(2):
                    nc.vector.tensor_tensor(xw[:, th * 512: th * 512 + 500],
                                            x[:, co * SP + th * 500: co * SP + (th + 1) * 500],
                                            ppw[:, th * 512: th * 512 + 500], ALU.mult)
                copy(xw[:, 500:512], zeros128[:, 0:12])
                copy(xw[:, 1012:1024], zeros128[:, 0:12])
                nc.vector.tensor_reduce(pooled[:, co:co + 1], xw[:], AX.X, ALU.add)
            # scale by 1/total
            rtotr = sb((1, 1), TR)
            copy(rtotr[:], rtot[:])
            prt = pt((128, 1), tag="pav")
            MMs(prt[:], o1x128r[:], rtotr[:], start=True, stop=True)
            rtb = sb((128, 1))
            copy(rtb[:], prt[:])
            nc.vector.tensor_scalar(pooled[:], pooled[:], rtb[:, 0:1], None, ALU.mult)
            # LN over the 256-vector
            poor = sb((128, EC), TR, tag="poor")
            copy(poor[:], pooled[:])
            poo2 = sb((128, EC), TR, tag="poo2")
            nc.vector.tensor_mul(poo2[:], pooled[:], pooled[:])
            pcs2 = pt((EC, 2), tag="pav")
            MMs(pcs2[:, 0:1], poor[:], ones128r[:], start=True, stop=True)
            MMs(pcs2[:, 1:2], poo2[:], ones128r[:], start=True, stop=True)
            cs2 = sb((EC, 2), TR)
            copy(cs2[:], pcs2[:])
            pcs3 = pt((2, 1), tag="pav")
            MMs(pcs3[:], cs2[:], onesECr[:], start=True, stop=True)
            cs3t = sb((2, 1), TR)
            copy(cs3t[:], pcs3[:])
            pcs4 = pt((1, 2), tag="pav")
            MMs(pcs4[:], cs3t[:], I4r[0:2, 0:2], start=True, stop=True)
            cs3 = sb((1, 2))
            nc.vector.tensor_scalar_mul(cs3[:], pcs4[:], 1.0 / 256.0)
            cm2 = sb((1, 1))
            nc.vector.tensor_mul(cm2[:], cs3[0:1, 0:1], cs3[0:1, 0:1])
            cvar = sb((1, 1))
            nc.vector.tensor_sub(cvar[:], cs3[0:1, 1:2], cm2[:])
            clnv = sb((1, 1))
            nc.scalar.activation(clnv[:], cvar[:], AF.Ln, bias=epsb[0:1, :])
            crstd = sb((1, 1), TR)
            nc.scalar.activation(crstd[:], clnv[:], AF.Exp, scale=-0.5)
            cmeanr = sb((1, 1), TR)
            copy(cmeanr[:], cs3[0:1, 0:1])
            pcb = pt((128, 2), tag="pav")
            MMs(pcb[:, 0:1], o1x128r[:], cmeanr[:], start=True, stop=True)
            MMs(pcb[:, 1:2], o1x128r[:], crstd[:], start=True, stop=True)
            yv = sb((128, EC), TR, tag="yv")
            for co in range(EC):
                t_ = sb((128, 1))
                nc.vector.tensor_sub(t_[:], pooled[:, co:co + 1], pcb[:, 0:1])
                nc.vector.tensor_scalar(t_[:], t_[:], pcb[:, 1:2], None, ALU.mult)
                nc.vector.tensor_scalar(yv[:, co:co + 1], t_[:], clng[:, co:co + 1],
                                        clnb[:, co:co + 1], ALU.mult, ALU.add)
            pz = pt((128, 1), tag="pav")
            for co in range(EC):
                MMs(pz[:], cw1T[:, co * 128:(co + 1) * 128], yv[:, co:co + 1],
                   start=(co == 0), stop=(co == EC - 1))
            zv = sb((128, 1), TR)
            nc.vector.tensor_scalar(zv[:], pz[:], cb1[:], None, ALU.add)
            nc.vector.tensor_scalar_max(zv[:].bitcast(TS), zv[:].bitcast(TS), 0.0)
            zv2 = sb((128, 1), TR)
            copy(zv2[:], zv[:].bitcast(TS))
            pout = pt((NCls, 1), tag="pav")
            MMs(pout[:], cw2T[:], zv2[:], start=True, stop=True)
            outv = sb((NCls, 1))
            nc.vector.tensor_scalar(outv[:], pout[:], cb2[:], None, ALU.add)
            nc.sync.dma_start(out_d, outv[:])

    nc.compile()
    return nc


_NC_CACHE = {}


def _host_inputs(inputs):
    I = {k: np.asarray(v, F32) for k, v in inputs.items()}
    h = {}
    h["embw1T"] = np.ascontiguousarray(I["emb_w1"].T).astype(B16)            # [5,128]
    h["embb1"] = I["emb_b1"].reshape(128, 1)

    def wT(w):
        O, II = w.shape
        return np.ascontiguousarray(w.T.reshape(II // 128, 128, O).transpose(1, 0, 2)).reshape(128, -1)

    h["embw2T"] = wT(I["emb_w2"]).astype(B16)                                # [128, 256]
    h["embb2"] = np.ascontiguousarray(I["emb_b2"].reshape(EC, 128).T)
    h["embln"] = np.concatenate([I["emb_ln_g"].reshape(EC, 128).T,
                                 I["emb_ln_b"].reshape(EC, 128).T], axis=1)
    posT = np.zeros((128, EC * SP), F32)
    pe = I["pos_enc"][:S]                                                    # [1000, 256]
    for co in range(EC):
        posT[:, co * SP: co * SP + S] = pe[:, co * 128:(co + 1) * 128].T
    h["posT"] = posT.astype(B16)
    for nm_, key in (("qwT", "qw"), ("kwT", "kw"), ("vwT", "vw"), ("owT", "ow")):
        h[nm_] = np.concatenate([wT(I[key][l]) for l in range(L)], axis=1).astype(B16)
    h["qkvb"] = np.concatenate(
        [np.concatenate([I["qb"][l].reshape(EC, 128).T, I["kb"][l].reshape(EC, 128).T,
                         I["vb"][l].reshape(EC, 128).T], axis=1) for l in range(L)], axis=1)
    h["obias"] = np.concatenate([I["ob"][l].reshape(EC, 128).T for l in range(L)], axis=1)
    h["vbrow"] = I["vb"].reshape(1, L * E).astype(B16)
    h["fw1T"] = np.concatenate([wT(I["f_w1"][l]) for l in range(L)], axis=1).astype(B16)
    h["fw2T"] = np.concatenate([wT(I["f_w2"][l]) for l in range(L)], axis=1).astype(B16)
    h["fb1"] = np.concatenate([I["f_b1"][l].reshape(HC, 128).T for l in range(L)], axis=1)
    h["fb2"] = np.concatenate([I["f_b2"][l].reshape(EC, 128).T for l in range(L)], axis=1)
    h["lng"] = np.concatenate([I["ln_g"][l].reshape(EC, 128).T for l in range(L)], axis=1)
    h["lnb"] = np.concatenate([I["ln_b"][l].reshape(EC, 128).T for l in range(L)], axis=1)
    h["pew1T"] = np.concatenate([np.ascontiguousarray(I["pe_w1"][l].T) for l in range(L)], axis=1)
    h["peb1"] = np.stack([I["pe_b1"][l] for l in range(L)], axis=1)
    h["pew2T"] = np.concatenate([wT(I["pe_w2"][l]) for l in range(L)], axis=1)
    h["peb2"] = np.concatenate([I["pe_b2"][l].reshape(EC, 128).T for l in range(L)], axis=1)
    h["clng"] = np.ascontiguousarray(I["c_ln_g"].reshape(EC, 128).T)
    h["clnb"] = np.ascontiguousarray(I["c_ln_b"].reshape(EC, 128).T)
    h["cw1T"] = wT(I["c_w1"])
    h["cb1"] = I["c_b1"].reshape(128, 1)
    h["cw2T"] = np.ascontiguousarray(I["c_w2"].T)                            # [128, 2]
    h["cb2"] = I["c_b2"].reshape(NCls, 1)
    h["ph_law"] = I["ph_law"].reshape(1, 2); h["ph_lab"] = I["ph_lab"].reshape(1, 1)
    h["ph_fw"] = I["ph_fw"].reshape(1, 1); h["ph_db"] = I["ph_db"].reshape(1, 1)
    tc_ = (np.arange(8)[None, :] * 128 + np.arange(128)[:, None]).astype(F32)
    h["tconst"] = tc_
    h["padneg"] = np.where(tc_ < S, F32(0), F32(-3e38)).astype(F32)
    h["vmask"] = (tc_ < S).astype(F32)
    h["iota50"] = np.broadcast_to(np.arange(50, dtype=F32), (128, 50)).copy()
    h["I50"] = np.eye(50, dtype=F32); h["maskD50"] = np.eye(50, dtype=F32)
    h["I4"] = np.eye(4, dtype=F32); h["I128"] = np.eye(128, dtype=F32)
    h["ones128"] = np.ones((128, 1), F32); h["ones50"] = np.ones((50, 1), F32)
    h["ones1x128"] = np.ones((1, 128), F32); h["ones1x50"] = np.ones((1, 50), F32)
    h["ones4"] = np.ones((4, 1), F32)
    h["onesEC"] = np.ones((EC, 1), F32)
    h["ones128b"] = np.ones((128, 1), B16); h["ones1x128b"] = np.ones((1, 128), B16)
    h["ones1x32b"] = np.ones((1, 32), B16)
    h["pwrT"] = np.ones((1, 128), B16)
    v0 = np.full((50, 1), 0.1414, F32); v0[::2, 0] *= -1
    h["v0"] = v0
    h["W0"] = (np.random.default_rng(1234).standard_normal((50, 4)).astype(F32) * F32(0.14))
    h["zeros128"] = np.zeros((128, 256), F32)
    h["epsb"] = np.full((128, 1), 1e-5, F32)
    vm = (tc_ < S).astype(F32)
    h["vmask8"] = np.repeat(vm, 8, axis=1)
    return h


def kernel(**inputs):
    if "nc" not in _NC_CACHE:
        _NC_CACHE["nc"] = build_nc()
    nc = _NC_CACHE["nc"]
    h = _host_inputs(inputs)
    seqs = np.asarray(inputs["sequences"], F32)
    in_maps = []
    for b in range(4):
        m = dict(h)
        seqp = np.zeros((SP, 5), F32)
        seqp[:S] = seqs[b]
        m["seqT5"] = np.ascontiguousarray(seqp.T).astype(B16)
        m["seqPH"] = np.ascontiguousarray(
            seqp.reshape(8, 128, 5).transpose(1, 0, 2).reshape(128, 40))
        in_maps.append(m)
    res = bass_utils.run_bass_kernel_spmd(nc, in_maps, core_ids=[0, 1, 2, 3])
    out = np.stack([res.results[b]["out"][:, 0] for b in range(4)], axis=0)
    return out.astype(np.float32)


# revision 23
# speedup vs baseline: 1.7465x; 1.1242x over previous
"""DeepTDATransformer TRN2 Bass kernel: 4-core sample-parallel, bf16 core."""
import numpy as np
import ml_dtypes
import concourse.bacc as bacc
import concourse.tile as tile
import concourse.mybir as mybir
from concourse import bass_utils

dt = mybir.dt
AF = mybir.ActivationFunctionType
ALU = mybir.AluOpType
AX = mybir.AxisListType
F32 = np.float32
B16 = np.float16
TS = dt.float32
TR = dt.float32r
TB = dt.float16

S, SP, E, H, DH, L, NCls = 1000, 1024, 256, 8, 32, 6, 2
EC = 2   # e chunks
HC = 8   # ffn hidden chunks

_uid = [0]


def _nm(p="i"):
    _uid[0] += 1
    return f"{p}{_uid[0]}"


def build_nc():
    nc = bacc.Bacc("TRN2", target_bir_lowering=False, debug=False, num_devices=4)
    d = {}

    def din(name, shape, dty=dt.float32):
        d[name] = nc.dram_tensor(name, list(shape), dty, kind="ExternalInput").ap()

    # bf16 transformer tensors
    din("seqT5", (5, SP), TB)
    din("embw1T", (5, 128), TB); din("embw2T", (128, EC * 128), TB)
    din("posT", (128, EC * SP), TB)
    din("qwT", (128, L * EC * E), TB); din("kwT", (128, L * EC * E), TB)
    din("vwT", (128, L * EC * E), TB); din("owT", (128, L * EC * E), TB)
    din("vbrow", (1, L * E), TB)
    din("fw1T", (128, L * EC * 1024), TB); din("fw2T", (128, L * HC * E), TB)
    din("pwrT", (1, 128), TB)  # bf16 ones row
    din("ones128b", (128, 1), TB); din("ones1x128b", (1, 128), TB)
    din("ones1x32b", (1, 32), TB)
    din("sel8", (128, 256), TB)
    # fp32 biases / scales (per-partition scalars)
    din("embb1", (128, 1)); din("embb2", (128, EC))
    din("embln", (128, 2 * EC))
    din("qkvb", (128, L * 3 * EC)); din("obias", (128, L * EC))
    din("fb1", (128, L * HC)); din("fb2", (128, L * EC))
    din("lng", (128, L * EC)); din("lnb", (128, L * EC))
    # PE-net (ts) fp32
    din("pew1T", (6, L * 128)); din("peb1", (128, L))
    din("pew2T", (128, L * E)); din("peb2", (128, L * EC))
    # classifier fp32
    din("clng", (128, EC)); din("clnb", (128, EC))
    din("cw1T", (128, EC * 128)); din("cb1", (128, 1))
    din("cw2T", (128, NCls)); din("cb2", (NCls, 1))
    # PH fp32
    din("seqPH", (128, 40))
    din("ph_law", (1, 2)); din("ph_lab", (1, 1)); din("ph_fw", (1, 1)); din("ph_db", (1, 1))
    din("tconst", (128, 8)); din("padneg", (128, 8)); din("vmask", (128, 8))
    din("iota50", (128, 50)); din("I50", (50, 50)); din("maskD50", (50, 50)); din("I4", (4, 4))
    din("I128", (128, 128)); din("ones128", (128, 1)); din("ones50", (50, 1))
    din("ones1x128", (1, 128)); din("ones1x50", (1, 50))
    din("ones4", (4, 1)); din("onesEC", (EC, 1))
    din("v0", (50, 1)); din("W0", (50, 4)); din("zeros128", (128, 256))
    din("epsb", (128, 1))
    din("vmask8", (128, 64))
    out_d = nc.dram_tensor("out", [NCls, 1], dt.float32, kind="ExternalOutput").ap()

    with tile.TileContext(nc) as tc:
        with (
            tc.tile_pool(name="const", bufs=1) as cp,
            tc.tile_pool(name="wp", bufs=2) as wp,
            tc.tile_pool(name="ap_", bufs=1) as app,
            tc.tile_pool(name="sm", bufs=2) as sm,
            tc.tile_pool(name="sm4", bufs=2) as sm4,
            tc.tile_pool(name="rp", bufs=3) as rp,
            tc.tile_pool(name="psA", bufs=2, space="PSUM") as psA,
            tc.tile_pool(name="psB", bufs=2, space="PSUM") as psB,
            tc.tile_pool(name="psC", bufs=2, space="PSUM") as psC,
        ):
            def c32(name, shape, nm=None, dty=TS):
                t = cp.tile(list(shape), dty, tag=nm or name, name=nm or name)
                nc.sync.dma_start(t[:], d[name])
                return t

            def c32r(name, shape, nm=None):
                t = cp.tile(list(shape), TR, tag=(nm or name) + "r", name=(nm or name) + "r")
                nc.gpsimd.dma_start(t[:], d[name])
                return t

            def cb16(name, shape, nm=None):
                t = cp.tile(list(shape), TB, tag=(nm or name) + "b", name=(nm or name) + "b")
                nc.sync.dma_start(t[:], d[name])
                return t

            # bf16 consts
            seqT5 = cb16("seqT5", (5, SP))
            embw1T = cb16("embw1T", (5, 128)); embw2T = cb16("embw2T", (128, EC * 128))
            posT = cb16("posT", (128, EC * SP))
            o1x128b = cb16("ones1x128b", (1, 128)); ones128b = cb16("ones128b", (128, 1))
            o1x32b = cb16("ones1x32b", (1, 32))
            sel8 = cb16("sel8", (128, 256))
            # fp32 consts
            embb1 = c32("embb1", (128, 1)); embb2 = c32("embb2", (128, EC))
            embln = c32("embln", (128, 2 * EC))
            seqPH = c32("seqPH", (128, 40))
            tconst = c32("tconst", (128, 8)); padneg = c32("padneg", (128, 8))
            iota50 = c32("iota50", (128, 50))
            I50r = c32r("I50", (50, 50)); maskD50 = c32("maskD50", (50, 50))
            I4 = c32("I4", (4, 4)); I4r = c32r("I4", (4, 4), "I4c"); I128r = c32r("I128", (128, 128))
            ones128r = c32r("ones128", (128, 1)); ones50r = c32r("ones50", (50, 1))
            o1x128r = c32r("ones1x128", (1, 128)); o1x50r = c32r("ones1x50", (1, 50))
            ones4r = c32r("ones4", (4, 1))
            onesECr = c32r("onesEC", (EC, 1))
            v0 = c32r("v0", (50, 1)); W0r = c32r("W0", (50, 4))
            zeros128 = c32("zeros128", (128, 256))
            vmask8 = c32("vmask8", (128, 64))
            epsb = c32("epsb", (128, 1))
            pew1T = c32r("pew1T", (6, L * 128)); peb1 = c32("peb1", (128, L))
            pew2T = c32r("pew2T", (128, L * E)); peb2 = c32("peb2", (128, L * EC))
            clng = c32("clng", (128, EC)); clnb = c32("clnb", (128, EC))
            cw1T = c32r("cw1T", (128, EC * 128)); cb1 = c32("cb1", (128, 1))
            cw2T = c32r("cw2T", (128, NCls)); cb2 = c32("cb2", (NCls, 1))
            law = c32r("ph_law", (1, 2)); lab = c32r("ph_lab", (1, 1))
            phfw = c32r("ph_fw", (1, 1)); phdb = c32r("ph_db", (1, 1))

            def pt(shape, tag="pj"):
                pool = {"sc": psA, "pj": psB, "pav": psC}[tag]
                return pool.tile(list(shape), TS, tag=tag, name=_nm("p"))

            def sb(shape, dtype=TS, pool=sm, tag=None, bufs=None):
                if tag is None:
                    fbytes = int(np.prod(shape[1:])) * mybir.dt.size(dtype)
                    if fbytes >= 2048:
                        return sm4.tile(list(shape), dtype, tag=f"g{fbytes}", name=_nm("s"))
                    tag = _nm("t")
                elif tag in ("row1k",):
                    return rp.tile(list(shape), dtype, tag=tag, name=_nm("s"))
                return pool.tile(list(shape), dtype, tag=tag, name=_nm("s"), bufs=bufs)

            def copy(dst, src):
                nc.vector.tensor_copy(dst, src)

            MM = nc.tensor.matmul

            def MMs(out, lhsT, rhs, **kw):
                l2 = lhsT.bitcast(TS) if lhsT.dtype == TR else lhsT
                r2 = rhs.bitcast(TS) if rhs.dtype == TR else rhs
                return MM(out, l2, r2, **kw)

            # ================= PH (fp32, unchanged math) =================
            mfeat = sb((128, 8))
            nc.vector.tensor_reduce(mfeat[:], seqPH[:].rearrange("p (c f) -> p c f", f=5),
                                    AX.X, ALU.add)
            nc.vector.tensor_scalar_mul(mfeat[:], mfeat[:], 0.2)
            p1 = pt((128, 8), tag="pav")
            MMs(p1[:, 0:2], o1x128r[:], law[:], start=True, stop=True)
            MMs(p1[:, 2:3], o1x128r[:], lab[:], start=True, stop=True)
            lawB = sb((128, 4))
            copy(lawB[:], p1[:, 0:4])
            scs = sb((128, 8))
            nc.vector.tensor_scalar(scs[:], tconst[:], lawB[:, 0:1], None, ALU.mult)
            tmp8 = sb((128, 8))
            nc.vector.tensor_scalar(tmp8[:], mfeat[:], lawB[:, 1:2], None, ALU.mult)
            nc.vector.tensor_add(scs[:], scs[:], tmp8[:])
            nc.vector.tensor_scalar(scs[:], scs[:], lawB[:, 2:3], None, ALU.add)
            nc.vector.tensor_add(scs[:], scs[:], padneg[:])
            scr = sb((128, 8), TR)
            copy(scr[:], scs[:])
            srow = sb((1, 1024), TR, tag="row1k")
            for hf in range(2):
                p2 = pt((1, 512), tag="pav")
                for c in range(4):
                    cc = hf * 4 + c
                    MMs(p2[:, c * 128:(c + 1) * 128], scr[:, cc:cc + 1], I128r[:],
                        start=True, stop=True)
                copy(srow[:, hf * 512:(hf + 1) * 512], p2[:])
            sROW = sb((128, 1024))
            for hh in range(2):
                p3 = pt((128, 512), tag="pav")
                MMs(p3[:], o1x128r[:], srow[:, hh * 512:(hh + 1) * 512],
                    start=True, stop=True)
                copy(sROW[:, hh * 512:(hh + 1) * 512], p3[:])
            rank = sb((128, 8))
            scratch = sb((128, 1024))
            for c in range(8):
                nc.vector.tensor_scalar(scratch[:], sROW[:], scs[:, c:c + 1], 0.0,
                                        ALU.is_gt, ALU.add, accum_out=rank[:, c:c + 1])
            ptsr = sb((128, 16), TR)
            pv = ptsr[:].rearrange("p (c two) -> p c two", two=2)
            copy(pv[:, :, 0:1], tconst[:].rearrange("p (c o) -> p c o", o=1))
            copy(pv[:, :, 1:2], mfeat[:].rearrange("p (c o) -> p c o", o=1))
            Gc = sb((128, 400), TR, tag="Gc", pool=app)
            for c in range(8):
                nc.vector.tensor_scalar(Gc[:, c * 50:(c + 1) * 50], iota50[:],
                                        rank[:, c:c + 1], None, ALU.is_equal)
            plm = pt((50, 2), tag="pav")
            plmT = pt((2, 50), tag="pav")
            for c in range(8):
                MMs(plm[:], Gc[:, c * 50:(c + 1) * 50], ptsr[:, c * 2:(c + 1) * 2],
                   start=(c == 0), stop=(c == 7))
            for c in range(8):
                MMs(plmT[:], ptsr[:, c * 2:(c + 1) * 2], Gc[:, c * 50:(c + 1) * 50],
                   start=(c == 0), stop=(c == 7))
            lmT = sb((2, 50), TR)
            copy(lmT[:], plmT[:])
            pg = pt((50, 50), tag="pav")
            MMs(pg[:], lmT[:], lmT[:], start=True, stop=True)
            gram = sb((50, 50))
            copy(gram[:], pg[:])
            sqd = sb((50, 50))
            nc.vector.tensor_mul(sqd[:], gram[:], maskD50[:])
            sq = sb((50, 1))
            nc.vector.tensor_reduce(sq[:], sqd[:], AX.X, ALU.add)
            t1 = sb((50, 50))
            nc.vector.tensor_scalar(t1[:], gram[:], -2.0, sq[:], ALU.mult, ALU.add)
            sqr = sb((50, 1), TR)
            copy(sqr[:], sq[:])
            p4 = pt((1, 50), tag="pav")
            MMs(p4[:], sqr[:], I50r[:], start=True, stop=True)
            sqrow = sb((1, 50), TR)
            copy(sqrow[:], p4[:])
            p5 = pt((50, 50), tag="pav")
            MMs(p5[:], o1x50r[:], sqrow[:], start=True, stop=True)
            d2 = sb((50, 50))
            nc.vector.tensor_add(d2[:], t1[:], p5[:])
            nc.vector.tensor_scalar_max(d2[:], d2[:], 1e-30)
            lnd = sb((50, 50))
            nc.scalar.activation(lnd[:], d2[:], AF.Ln)
            distm = sb((50, 50))
            nc.scalar.activation(distm[:], lnd[:], AF.Exp, scale=0.5)
            p6 = pt((50, 2), tag="pav")
            MMs(p6[:, 0:1], o1x50r[:], phfw[:], start=True, stop=True)
            MMs(p6[:, 1:2], o1x50r[:], phdb[:], start=True, stop=True)
            fwdb = sb((50, 2))
            copy(fwdb[:], p6[:])
            nfw = sb((50, 2))
            nc.scalar.activation(nfw[:, 0:1], fwdb[:, 0:1], AF.Abs)
            nc.vector.tensor_scalar_mul(nfw[:, 1:2], fwdb[:, 1:2], -1.0)
            dists = sb((50, 50))
            nc.vector.tensor_scalar(dists[:], distm[:], nfw[:, 0:1], None, ALU.mult)
            Km = sb((50, 50))
            nc.scalar.activation(Km[:], dists[:], AF.Exp, scale=-1.0, bias=nfw[:, 1:2])
            s_r = sb((50, 1))
            nc.vector.tensor_reduce(s_r[:], Km[:], AX.X, ALU.add)
            Bm = sb((50, 50))
            nc.vector.tensor_scalar(Bm[:], maskD50[:], s_r[:], None, ALU.mult)
            nc.vector.tensor_sub(Bm[:], Bm[:], Km[:])
            nc.vector.tensor_scalar_mul(Bm[:], Bm[:], -1.0)
            D40 = sb((50, 50))
            nc.vector.tensor_scalar_mul(D40[:], maskD50[:], 40.0)
            nc.vector.tensor_add(Bm[:], Bm[:], D40[:])
            nc.vector.tensor_scalar_add(Bm[:], Bm[:], -0.8)
            Br = sb((50, 50), TR)
            copy(Br[:], Bm[:])

            def vec_norm(vr):
                pn = pt((1, 1), tag="pav")
                MMs(pn[:], vr[:], vr[:], start=True, stop=True)
                lnv = sb((1, 1))
                nc.scalar.activation(lnv[:], pn[:], AF.Ln)
                rs = sb((1, 1), TR)
                nc.scalar.activation(rs[:], lnv[:], AF.Exp, scale=-0.5)
                prb = pt((50, 1), tag="pav")
                MMs(prb[:], o1x50r[:], rs[:], start=True, stop=True)
                vn = sb((50, 1), TR, tag="vpow")
                nc.vector.tensor_mul(vn[:].bitcast(TS), vr[:].bitcast(TS), prb[:])
                vn2 = sb((50, 1), TR, tag="vpow")
                copy(vn2[:], vn[:].bitcast(TS))
                return vn2

            v = v0
            for it in range(12):
                pv_ = pt((50, 1), tag="pav")
                MMs(pv_[:], Br[:], v[:], start=True, stop=True)
                v = sb((50, 1), TR, tag="vpow")
                nc.vector.tensor_scalar_mul(v[:], pv_[:], 0.125)
                if it % 4 == 3:
                    v = vec_norm(v)
            v = vec_norm(v)
            pbv = pt((50, 1), tag="pav")
            MMs(pbv[:], Br[:], v[:], start=True, stop=True)
            vbvf = sb((50, 1))
            nc.vector.tensor_mul(vbvf[:], v[:].bitcast(TS), pbv[:])
            vbv = sb((50, 1), TR)
            copy(vbv[:], vbvf[:])
            pmu = pt((1, 1), tag="pav")
            MMs(pmu[:], vbv[:], ones50r[:], start=True, stop=True)
            mu1 = sb((1, 1))
            copy(mu1[:], pmu[:])
            pvr = pt((1, 50), tag="pav")
            MMs(pvr[:], v[:], I50r[:], start=True, stop=True)
            vRow = sb((1, 50), TR)
            copy(vRow[:], pvr[:])

            def ns_orth(W, nstep):
                pg_ = pt((4, 4), tag="pav")
                MMs(pg_[:], W[:], W[:], start=True, stop=True)
                gd = sb((4, 4))
                nc.vector.tensor_mul(gd[:], pg_[:], I4[:])
                gdr = sb((4, 1))
                nc.vector.tensor_reduce(gdr[:], gd[:], AX.X, ALU.add)
                gdr2 = sb((4, 1), TR)
                copy(gdr2[:], gdr[:])
                ptr = pt((1, 1), tag="pav")
                MMs(ptr[:], gdr2[:], ones4r[:], start=True, stop=True)
                lnt = sb((1, 1))
                nc.scalar.activation(lnt[:], ptr[:], AF.Ln, scale=0.25)
                rst = sb((1, 1), TR)
                nc.scalar.activation(rst[:], lnt[:], AF.Exp, scale=-0.5)
                prb = pt((50, 1), tag="pav")
                MMs(prb[:], o1x50r[:], rst[:], start=True, stop=True)
                Wn = sb((50, 4), TR, tag="Wsub")
                nc.vector.tensor_scalar(Wn[:], W[:].bitcast(TS), prb[:], None, ALU.mult)
                W = Wn
                for _ in range(nstep):
                    pg2 = pt((4, 4), tag="pav")
                    MMs(pg2[:], W[:], W[:], start=True, stop=True)
                    i4h = sb((4, 4))
                    nc.vector.tensor_scalar_mul(i4h[:], I4[:], 1.5)
                    corrf = sb((4, 4))
                    nc.vector.tensor_scalar(corrf[:], pg2[:], -0.5, None, ALU.mult)
                    corr = sb((4, 4), TR)
                    nc.vector.tensor_add(corr[:], corrf[:], i4h[:])
                    pwt = pt((4, 50), tag="pav")
                    MMs(pwt[:], W[:], I50r[:], start=True, stop=True)
                    WT = sb((4, 50), TR)
                    copy(WT[:], pwt[:])
                    pw2 = pt((50, 4), tag="pav")
                    MMs(pw2[:], WT[:], corr[:], start=True, stop=True)
                    W = sb((50, 4), TR, tag="Wsub")
                    copy(W[:], pw2[:])
                return W

            W = W0r
            for it in range(14):
                pw_ = pt((50, 4), tag="pav")
                MMs(pw_[:], Br[:], W[:], start=True, stop=True)
                Wn = sb((50, 4), TR, tag="Wsub")
                nc.vector.tensor_scalar_mul(Wn[:], pw_[:], 0.125)
                W = Wn
                pc_ = pt((1, 4), tag="pav")
                MMs(pc_[:], v[:], W[:], start=True, stop=True)
                cvw = sb((1, 4), TR)
                copy(cvw[:], pc_[:])
                pcor = pt((50, 4), tag="pav")
                MMs(pcor[:], vRow[:], cvw[:], start=True, stop=True)
                Wn = sb((50, 4), TR, tag="Wsub")
                nc.vector.tensor_sub(Wn[:].bitcast(TS), W[:].bitcast(TS), pcor[:])
                W2_ = sb((50, 4), TR, tag="Wsub")
                copy(W2_[:], Wn[:].bitcast(TS))
                W = W2_
                if it % 6 == 5:
                    W = ns_orth(W, 3)
            W = ns_orth(W, 6)
            pbw = pt((50, 4), tag="pav")
            MMs(pbw[:], Br[:], W[:], start=True, stop=True)
            BW = sb((50, 4), TR)
            copy(BW[:], pbw[:])
            ph4 = pt((4, 4), tag="pav")
            MMs(ph4[:], W[:], BW[:], start=True, stop=True)
            H4 = sb((4, 4))
            copy(H4[:], ph4[:])
            h4d = sb((4, 4)); h4f = sb((4, 4))
            nc.vector.tensor_mul(h4d[:], H4[:], I4[:])
            nc.vector.tensor_mul(h4f[:], H4[:], H4[:])
            rd = sb((4, 1)); rf = sb((4, 1))
            nc.vector.tensor_reduce(rd[:], h4d[:], AX.X, ALU.add)
            nc.vector.tensor_reduce(rf[:], h4f[:], AX.X, ALU.add)
            rdr = sb((4, 2), TR)
            copy(rdr[:, 0:1], rd[:]); copy(rdr[:, 1:2], rf[:])
            pst_ = pt((2, 1), tag="pav")
            MMs(pst_[:], rdr[:], ones4r[:], start=True, stop=True)
            stt2 = sb((2, 1), TR)
            copy(stt2[:], pst_[:])
            pstr = pt((1, 2), tag="pav")
            MMs(pstr[:], stt2[:], I4r[0:2, 0:2], start=True, stop=True)
            sttrow = sb((1, 2))
            copy(sttrow[:], pstr[:])
            frH0 = sttrow[0:1, 1:2]
            mean_mu = sb((1, 1))
            nc.vector.tensor_scalar_mul(mean_mu[:], sttrow[0:1, 0:1], 0.25)
            m2 = sb((1, 1))
            nc.vector.tensor_mul(m2[:], mean_mu[:], mean_mu[:])
            nc.vector.tensor_scalar_mul(m2[:], m2[:], -4.0 / 3.0)
            varq = sb((1, 1))
            nc.vector.tensor_scalar_mul(varq[:], frH0[:], 1.0 / 3.0)
            nc.vector.tensor_add(varq[:], varq[:], m2[:])
            nc.vector.tensor_scalar_max(varq[:], varq[:], 1e-30)
            lnv3 = sb((1, 1))
            nc.scalar.activation(lnv3[:], varq[:], AF.Ln)
            std_ev = sb((1, 1))
            nc.scalar.activation(std_ev[:], lnv3[:], AF.Exp, scale=0.5)
            mean_ev = sb((1, 1))
            nc.vector.tensor_scalar(mean_ev[:], mean_mu[:], -1.0, 40.0, ALU.mult, ALU.add)
            gap = sb((1, 1))
            nc.vector.tensor_scalar(gap[:], mu1[:], -1.0, 40.0, ALU.mult, ALU.add)
            pfrow = sb((1, 8))
            copy(pfrow[:], zeros128[0:1, 0:8])
            nc.vector.tensor_scalar_add(pfrow[:, 0:1], pfrow[:, 0:1], 1.0)
            nc.vector.tensor_scalar_add(pfrow[:, 3:4], pfrow[:, 3:4], 1.0 / 7.0)
            copy(pfrow[:, 2:3], gap[:])
            copy(pfrow[:, 4:5], mean_ev[:])
            copy(pfrow[:, 5:6], std_ev[:])
            pfrr = sb((1, 8), TR)
            copy(pfrr[:], pfrow[:])
            ppf = pt((8, 1), tag="pav")
            MMs(ppf[:], pfrr[:], o1x128r[:, 0:1], start=True, stop=True)
            pfr = sb((8, 1), TR)
            copy(pfr[:], ppf[:])
            pfr = pfr[0:6, :]

            # ts per layer (fp32 PE-net)
            tsB = sb((128, L), tag="tsB", pool=app)
            for l in range(L):
                ph1 = pt((128, 1), tag="pav")
                MMs(ph1[:], pew1T[:, l * 128:(l + 1) * 128], pfr[:], start=True, stop=True)
                h1f = sb((128, 1))
                nc.vector.tensor_scalar(h1f[:], ph1[:], peb1[:, l:l + 1], None, ALU.add)
                h1b = sb((128, 1), TR)
                nc.vector.tensor_scalar_max(h1b[:], h1f[:], 0.0)
                sig = sb((128, EC))
                for co in range(EC):
                    py = pt((128, 1), tag="pav")
                    MMs(py[:], pew2T[:, (l * EC + co) * 128:(l * EC + co + 1) * 128],
                       h1b[:], start=True, stop=True)
                    yb = sb((128, 1))
                    nc.vector.tensor_scalar(yb[:], py[:], peb2[:, l * EC + co:l * EC + co + 1],
                                            None, ALU.add)
                    ey = sb((128, 1))
                    nc.scalar.activation(ey[:], yb[:], AF.Exp, scale=-1.0)
                    nc.vector.tensor_scalar_add(ey[:], ey[:], 1.0)
                    lt_ = sb((128, 1))
                    nc.scalar.activation(lt_[:], ey[:], AF.Ln)
                    nc.scalar.activation(sig[:, co:co + 1], lt_[:], AF.Exp, scale=-1.0)
                sigr = sb((128, EC), TR)
                copy(sigr[:], sig[:])
                pts_ = pt((EC, 1), tag="pav")
                MMs(pts_[:], sigr[:], ones128r[:], start=True, stop=True)
                tsum = sb((EC, 1), TR)
                copy(tsum[:], pts_[:])
                pt2 = pt((1, 1), tag="pav")
                MMs(pt2[:], tsum[:], onesECr[:], start=True, stop=True)
                tsv = sb((1, 1), TR)
                nc.vector.tensor_scalar_mul(tsv[:], pt2[:], float(1.0 / (256.0 * np.sqrt(32.0))))
                ptb = pt((128, 1), tag="pav")
                MMs(ptb[:], o1x128r[:], tsv[:], start=True, stop=True)
                copy(tsB[:, l:l + 1], ptb[:])

            # ================= embedding (bf16) =================
            e1r = sb((128, SP), TB, tag="att", pool=app)
            for th in range(2):
                pe_ = pt((128, 512), tag="pj")
                MM(pe_[:, 0:500], embw1T[:], seqT5[:, th * 500:(th + 1) * 500],
                   start=True, stop=True)
                nc.scalar.activation(e1r[:, th * 500:(th + 1) * 500], pe_[:, 0:500],
                                     AF.Relu, bias=embb1[:, 0:1])
            xemb = sb((128, EC * SP), TB, tag="resid", pool=app)
            for co in range(EC):
                for th in range(2):
                    px = pt((128, 512), tag="pj")
                    MM(px[:, 0:500], embw2T[:, co * 128:(co + 1) * 128],
                       e1r[:, th * 500:th * 500 + 500], start=True, stop=True)
                    nc.vector.tensor_scalar(xemb[:, co * SP + th * 500: co * SP + (th + 1) * 500],
                                            px[:, 0:500], embb2[:, co:co + 1], None, ALU.add)
                nc.gpsimd.tensor_copy(xemb[:, co * SP + 1000: co * SP + 1024],
                                      zeros128[:, 0:24])

            def ln_T(xr, g_fn, b_fn, extra_fn=None):
                # xr: bf16 [128, EC*SP] -> returns bf16 [128, EC*SP] LN'd
                x2 = sb((128, EC * SP), TB, tag="att", pool=app)
                for co in range(EC):
                    nc.vector.tensor_tensor(x2[:, co * SP:co * SP + 1000],
                                            xr[:, co * SP:co * SP + 1000],
                                            xr[:, co * SP:co * SP + 1000], ALU.mult)
                pstS = pt((1, 1024), tag="sc")
                pstQ = pt((1, 1024), tag="sc")
                for th in range(2):
                    for co in range(EC):
                        MM(pstS[0:1, th * 512:th * 512 + 500], ones128b[:],
                           xr[:, co * SP + th * 500: co * SP + (th + 1) * 500],
                           start=(co == 0), stop=(co == EC - 1))
                    for co in range(EC):
                        MM(pstQ[0:1, th * 512:th * 512 + 500], ones128b[:],
                           x2[:, co * SP + th * 500: co * SP + (th + 1) * 500],
                           start=(co == 0), stop=(co == EC - 1))
                mu_b = sb((1, 1024), TB, tag="rowb", bufs=2)
                nc.vector.tensor_scalar_mul(mu_b[:], pstS[:], 1.0 / 256.0)
                e2 = sb((1, 1024), TS, tag="row1k")
                nc.vector.tensor_scalar_mul(e2[:], pstQ[:], 1.0 / 256.0)
                m2_ = sb((1, 1024), TS, tag="row1k")
                nc.gpsimd.tensor_tensor(m2_[:], mu_b[:], mu_b[:], ALU.mult)
                var = sb((1, 1024), TS, tag="row1k")
                nc.vector.tensor_tensor(var[:], e2[:], m2_[:], ALU.subtract)
                lnv_ = sb((1, 1024), TS, tag="row1k")
                nc.scalar.activation(lnv_[:], var[:], AF.Ln, bias=epsb[0:1, :])
                rstd_b = sb((1, 1024), TB, tag="rowb", bufs=2)
                nc.scalar.activation(rstd_b[:], lnv_[:], AF.Exp, scale=-0.5)
                pmb = pt((128, 1024), tag="sc")
                prb = pt((128, 1024), tag="sc")
                for hh in range(2):
                    MM(pmb[:, hh * 512:(hh + 1) * 512], o1x128b[:],
                       mu_b[:, hh * 512:(hh + 1) * 512], start=True, stop=True)
                for hh in range(2):
                    MM(prb[:, hh * 512:(hh + 1) * 512], o1x128b[:],
                       rstd_b[:, hh * 512:(hh + 1) * 512], start=True, stop=True)
                prbS = sb((128, 1024), TB, tag="prbS", bufs=1)
                nc.vector.tensor_copy(prbS[:], prb[:])
                out = sb((128, EC * SP), TB, tag="x_ln", pool=app)
                for co in range(EC):
                    for th in range(2):
                        xs = xr[:, co * SP + th * 500: co * SP + (th + 1) * 500]
                        ms = pmb[:, th * 512: th * 512 + 500]
                        rs = prbS[:, th * 512: th * 512 + 500]
                        os_ = out[:, co * SP + th * 500: co * SP + (th + 1) * 500]
                        xc = sb((128, 512), TB, tag="xc")
                        nc.vector.tensor_tensor(xc[:, 0:500], xs, ms, ALU.subtract)
                        y = sb((128, 512), TB, tag="yln")
                        nc.gpsimd.tensor_tensor(y[:, 0:500], xc[:, 0:500], rs, ALU.mult)
                        if extra_fn is None:
                            nc.vector.tensor_scalar(os_, y[:, 0:500], g_fn(co), b_fn(co),
                                                    ALU.mult, ALU.add)
                        else:
                            t2 = sb((128, 512), TB, tag="t2ln")
                            nc.vector.tensor_scalar(t2[:, 0:500], y[:, 0:500],
                                                    g_fn(co), b_fn(co), ALU.mult, ALU.add)
                            nc.gpsimd.tensor_tensor(os_, t2[:, 0:500], extra_fn(co, th),
                                                    ALU.add)
                    nc.gpsimd.tensor_copy(out[:, co * SP + 1000: co * SP + 1024],
                                          zeros128[:, 0:24])
                return out

            x = ln_T(xemb,
                     lambda co: embln[:, co:co + 1], lambda co: embln[:, EC + co:EC + co + 1],
                     extra_fn=lambda co, th: posT[:, co * SP + th * 500: co * SP + (th + 1) * 500])

            # Vtm one-time init: mask col + zero pad-key rows
            Vtm = sb((128, 8 * 264), TB, tag="Vtm", pool=app)
            vslice = Vtm[:].rearrange("p (tc h c) -> p tc h c", tc=8, h=H)
            for tcb in range(8):
                nc.gpsimd.tensor_copy(
                    vslice[:, tcb, :, 32:33],
                    vmask8[:, tcb * 8:(tcb + 1) * 8].rearrange("p (h o) -> p h o", o=1))
            nc.gpsimd.tensor_copy(
                vslice[96:128, 7, :, 0:32],
                zeros128[0:32, 0:256].rearrange("p (h dd) -> p h dd", h=H))
            denBt = [sb((128, 1024), TB, tag=f"denBt{g}", bufs=1) for g in range(2)]
            for g in range(2):
                nc.vector.memset(denBt[g][:], 1.0)

            # ================= layers =================
            for l in range(L):
                wq = wp.tile([128, EC * E], TB, tag="wq", name=_nm("wq"))
                nc.sync.dma_start(wq[:], d["qwT"][:, l * EC * E:(l + 1) * EC * E])
                wk = wp.tile([128, EC * E], TB, tag="wk", name=_nm("wk"))
                nc.sync.dma_start(wk[:], d["kwT"][:, l * EC * E:(l + 1) * EC * E])
                wv = wp.tile([128, EC * E], TB, tag="wv", name=_nm("wv"))
                nc.sync.dma_start(wv[:], d["vwT"][:, l * EC * E:(l + 1) * EC * E])
                wo = wp.tile([128, EC * E], TB, tag="wo", name=_nm("wo"))
                nc.sync.dma_start(wo[:], d["owT"][:, l * EC * E:(l + 1) * EC * E])
                w1 = wp.tile([128, EC * 1024], TB, tag="w1", name=_nm("w1"))
                nc.sync.dma_start(w1[:], d["fw1T"][:, l * EC * 1024:(l + 1) * EC * 1024])
                w2 = wp.tile([128, HC * E], TB, tag="w2", name=_nm("w2"))
                nc.sync.dma_start(w2[:], d["fw2T"][:, l * HC * E:(l + 1) * HC * E])
                vbr = wp.tile([1, E], TB, tag="vbr", name=_nm("vbr"))
                nc.sync.dma_start(vbr[:], d["vbrow"][:, l * E:(l + 1) * E])
                bq = wp.tile([128, 3 * EC], TS, tag="bqkv", name=_nm("bq"))
                nc.sync.dma_start(bq[:], d["qkvb"][:, l * 3 * EC:(l + 1) * 3 * EC])
                bo = wp.tile([128, EC], TS, tag="bo", name=_nm("bo"))
                nc.sync.dma_start(bo[:], d["obias"][:, l * EC:(l + 1) * EC])
                b1 = wp.tile([128, HC], TS, tag="b1", name=_nm("b1"))
                nc.sync.dma_start(b1[:], d["fb1"][:, l * HC:(l + 1) * HC])
                b2 = wp.tile([128, EC], TS, tag="b2", name=_nm("b2"))
                nc.sync.dma_start(b2[:], d["fb2"][:, l * EC:(l + 1) * EC])
                lg = wp.tile([128, EC], TS, tag="lg", name=_nm("lg"))
                nc.sync.dma_start(lg[:], d["lng"][:, l * EC:(l + 1) * EC])
                lb = wp.tile([128, EC], TS, tag="lb", name=_nm("lb"))
                nc.sync.dma_start(lb[:], d["lnb"][:, l * EC:(l + 1) * EC])

                # ---- Q, K projections (bf16) ----
                qTs = sb((128, EC * SP), TB, tag="qTs", pool=app)
                kT = sb((128, EC * SP), TB, tag="kT", pool=app)
                for (wt, outt, bofs) in ((wq, qTs, 0), (wk, kT, EC)):
                    for co in range(EC):
                        for th in range(2):
                            pp = pt((128, 512), tag="pj")
                            for ci in range(EC):
                                MM(pp[:, 0:500],
                                   wt[:, (ci * EC + co) * 128:(ci * EC + co + 1) * 128],
                                   x[:, ci * SP + th * 500: ci * SP + (th + 1) * 500],
                                   start=(ci == 0), stop=(ci == EC - 1))
                            sl = outt[:, co * SP + th * 500: co * SP + (th + 1) * 500]
                            nc.vector.tensor_scalar(sl, pp[:, 0:500],
                                                    bq[:, bofs + co: bofs + co + 1],
                                                    None, ALU.add)
                        nc.gpsimd.tensor_copy(outt[:, co * SP + 1000: co * SP + 1024],
                                              zeros128[:, 0:24])

                # ---- V (token-major, bf16) ----
                for tcb in range(8):
                    pv2 = pt((128, 512), tag="pj")
                    for ci in range(EC):
                        MM(pv2[:, 0:256],
                           x[:, ci * SP + tcb * 128: ci * SP + (tcb + 1) * 128],
                           wv[:, ci * E:(ci + 1) * E],
                           start=(ci == 0), stop=False)
                    MM(pv2[:, 0:256], o1x128b[:], vbr[:], start=False, stop=True)
                    nrows = 128 if tcb < 7 else 104
                    nc.vector.tensor_copy(
                        vslice[0:nrows, tcb, :, 0:32],
                        pv2[0:nrows, 0:256].rearrange("p (h dd) -> p h dd", h=H))

                # ---- attention per head ----
                att = sb((128, EC * SP), TB, tag="att", pool=app)
                uais = []
                for hh in range(H):
                    co_h, r0 = hh // 4, (hh % 4) * 32
                    expsT = sb((128, 8 * SP), TB, tag="expsT", pool=app, bufs=2)
                    for kc in range(8):
                        psc = pt((128, 1024), tag="sc")
                        for qh in range(2):
                            MM(psc[:, qh * 512:(qh + 1) * 512],
                               kT[r0:r0 + 32, co_h * SP + kc * 128: co_h * SP + (kc + 1) * 128],
                               qTs[r0:r0 + 32, co_h * SP + qh * 512: co_h * SP + (qh + 1) * 512],
                               start=True, stop=True, tile_position=(r0, 0))
                        nc.scalar.activation(expsT[:, kc * 1024:(kc + 1) * 1024], psc[:],
                                             AF.Exp, scale=tsB[:, l:l + 1])
                    uai = sb((33, 1024), TB, tag="uai", bufs=8)
                    for qh in range(2):
                        pav = pt((33, 512), tag="pav")
                        for kc in range(8):
                            MM(pav[:],
                               Vtm[:, kc * 264 + hh * 33: kc * 264 + (hh + 1) * 33],
                               expsT[:, kc * 1024 + qh * 512: kc * 1024 + (qh + 1) * 512],
                               start=(kc == 0), stop=(kc == 7))
                        nc.vector.tensor_copy(uai[:, qh * 512:(qh + 1) * 512], pav[:])
                    nc.gpsimd.tensor_copy(denBt[co_h][r0:r0 + 1, :], uai[32:33, :])
                    uais.append(uai)
                # batched softmax denominators: 1/x via Ln+Exp on ACT (nle table)
                rdenbs = []
                for g in range(2):
                    lnD = sb((128, 1024), TS, tag="row1k")
                    nc.scalar.activation(lnD[:], denBt[g][:], AF.Ln)
                    rdenb = sb((128, 1024), TB, tag=f"rdenb{g}", bufs=1)
                    nc.scalar.activation(rdenb[:], lnD[:], AF.Exp, scale=-1.0)
                    rdenbs.append(rdenb)
                for hh in range(H):
                    co_h, r0 = hh // 4, (hh % 4) * 32
                    for qh in range(2):
                        prr = pt((32, 512), tag="pav")
                        MM(prr[:], sel8[:, hh * 32:(hh + 1) * 32],
                           rdenbs[co_h][:, qh * 512:(qh + 1) * 512], start=True, stop=True)
                        nc.vector.tensor_tensor(
                            att[r0:r0 + 32, co_h * SP + qh * 512: co_h * SP + (qh + 1) * 512],
                            uais[hh][0:32, qh * 512:(qh + 1) * 512], prr[:], ALU.mult)

                # ---- O proj + residual ----
                resid = sb((128, EC * SP), TB, tag="resid", pool=app)
                for co in range(EC):
                    for th in range(2):
                        po = pt((128, 512), tag="pj")
                        for ci in range(EC):
                            MM(po[:, 0:500],
                               wo[:, (ci * EC + co) * 128:(ci * EC + co + 1) * 128],
                               att[:, ci * SP + th * 500: ci * SP + (th + 1) * 500],
                               start=(ci == 0), stop=(ci == EC - 1))
                        tbo = sb((128, 512), TB, tag="tbo")
                        nc.vector.tensor_scalar(tbo[:, 0:500], po[:, 0:500],
                                                bo[:, co:co + 1], None, ALU.add)
                        sl = resid[:, co * SP + th * 500: co * SP + (th + 1) * 500]
                        nc.gpsimd.tensor_tensor(sl, tbo[:, 0:500],
                                                x[:, co * SP + th * 500: co * SP + (th + 1) * 500],
                                                ALU.add)
                    nc.gpsimd.tensor_copy(resid[:, co * SP + 1000: co * SP + 1024],
                                          zeros128[:, 0:24])
                x = ln_T(resid,
                         lambda co, lg=lg: lg[:, co:co + 1], lambda co, lb=lb: lb[:, co:co + 1])

                # ---- FFN ----
                resid2 = sb((128, EC * SP), TB, tag="resid", pool=app)
                for th in range(2):
                    hR = sb((128, HC * 512), TB, tag="hR", pool=app, bufs=2)
                    for hc in range(HC):
                        pf_ = pt((128, 512), tag="pj")
                        for ci in range(EC):
                            MM(pf_[:, 0:500],
                               w1[:, (ci * HC + hc) * 128:(ci * HC + hc + 1) * 128],
                               x[:, ci * SP + th * 500: ci * SP + (th + 1) * 500],
                               start=(ci == 0), stop=(ci == EC - 1))
                        nc.scalar.activation(hR[:, hc * 512: hc * 512 + 500],
                                             pf_[:, 0:500], AF.Gelu, bias=b1[:, hc:hc + 1])
                    for co in range(EC):
                        p2_ = pt((128, 512), tag="pj")
                        for hc in range(HC):
                            MM(p2_[:, 0:500],
                               w2[:, (hc * EC + co) * 128:(hc * EC + co + 1) * 128],
                               hR[:, hc * 512: hc * 512 + 500],
                               start=(hc == 0), stop=(hc == HC - 1))
                        tb2 = sb((128, 512), TB, tag="tbo")
                        nc.vector.tensor_scalar(tb2[:, 0:500], p2_[:, 0:500],
                                                b2[:, co:co + 1], None, ALU.add)
                        sl = resid2[:, co * SP + th * 500: co * SP + (th + 1) * 500]
                        nc.gpsimd.tensor_tensor(sl, tb2[:, 0:500],
                                                x[:, co * SP + th * 500: co * SP + (th + 1) * 500],
                                                ALU.add)
                for co in range(EC):
                    nc.gpsimd.tensor_copy(resid2[:, co * SP + 1000: co * SP + 1024],
                                          zeros128[:, 0:24])
                x = ln_T(resid2,
                         lambda co, lg=lg: lg[:, co:co + 1], lambda co, lb=lb: lb[:, co:co + 1])

            # ================= pooling + classifier =================
            pcs = pt((1, 1024), tag="sc")
            for co in range(EC):
                for th in range(2):
                    MM(pcs[0:1, th * 512: th * 512 + 500], ones128b[:],
                       x[:, co * SP + th * 500: co * SP + (th + 1) * 500],
                       start=(co == 0), stop=(co == EC - 1))
            pwacc = sb((1, 2), tag="pwacc")
            pwr = sb((1, 1024), TB, tag="pwrb")
            for th in range(2):
                nc.scalar.activation(pwr[:, th * 512: th * 512 + 500],
                                     pcs[:, th * 512: th * 512 + 500], AF.Exp,
                                     accum_out=pwacc[:, th:th + 1])
            tot = sb((1, 1))
            nc.vector.tensor_add(tot[:], pwacc[:, 0:1], pwacc[:, 1:2])
            rtot = sb((1, 1))
            nc.vector.reciprocal(rtot[:], tot[:])
            pooled = sb((128, EC), tag="pooled")
            ppw = pt((128, 1024), tag="sc")
            for th in range(2):
                MM(ppw[:, th * 512:(th + 1) * 512], o1x128b[:],
                   pwr[:, th * 512:(th + 1) * 512], start=True, stop=True)
            for co in range(EC):
                xw = sb((128, 1024))
                for th in range(2):
                    nc.vector.tensor_tensor(xw[:, th * 512: th * 512 + 500],
                                            x[:, co * SP + th * 500: co * SP + (th + 1) * 500],
                                            ppw[:, th * 512: th * 512 + 500], ALU.mult)
                copy(xw[:, 500:512], zeros128[:, 0:12])
                copy(xw[:, 1012:1024], zeros128[:, 0:12])
                nc.vector.tensor_reduce(pooled[:, co:co + 1], xw[:], AX.X, ALU.add)
            # scale by 1/total
            rtotr = sb((1, 1), TR)
            copy(rtotr[:], rtot[:])
            prt = pt((128, 1), tag="pav")
            MMs(prt[:], o1x128r[:], rtotr[:], start=True, stop=True)
            rtb = sb((128, 1))
            copy(rtb[:], prt[:])
            nc.vector.tensor_scalar(pooled[:], pooled[:], rtb[:, 0:1], None, ALU.mult)
            # LN over the 256-vector
            poor = sb((128, EC), TR, tag="poor")
            copy(poor[:], pooled[:])
            poo2 = sb((128, EC), TR, tag="poo2")
            nc.vector.tensor_mul(poo2[:], pooled[:], pooled[:])
            pcs2 = pt((EC, 2), tag="pav")
            MMs(pcs2[:, 0:1], poor[:], ones128r[:], start=True, stop=True)
            MMs(pcs2[:, 1:2], poo2[:], ones128r[:], start=True, stop=True)
            cs2 = sb((EC, 2), TR)
            copy(cs2[:], pcs2[:])
            pcs3 = pt((2, 1), tag="pav")
            MMs(pcs3[:], cs2[:], onesECr[:], start=True, stop=True)
            cs3t = sb((2, 1), TR)
            copy(cs3t[:], pcs3[:])
            pcs4 = pt((1, 2), tag="pav")
            MMs(pcs4[:], cs3t[:], I4r[0:2, 0:2], start=True, stop=True)
            cs3 = sb((1, 2))
            nc.vector.tensor_scalar_mul(cs3[:], pcs4[:], 1.0 / 256.0)
            cm2 = sb((1, 1))
            nc.vector.tensor_mul(cm2[:], cs3[0:1, 0:1], cs3[0:1, 0:1])
            cvar = sb((1, 1))
            nc.vector.tensor_sub(cvar[:], cs3[0:1, 1:2], cm2[:])
            clnv = sb((1, 1))
            nc.scalar.activation(clnv[:], cvar[:], AF.Ln, bias=epsb[0:1, :])
            crstd = sb((1, 1), TR)
            nc.scalar.activation(crstd[:], clnv[:], AF.Exp, scale=-0.5)
            cmeanr = sb((1, 1), TR)
            copy(cmeanr[:], cs3[0:1, 0:1])
            pcb = pt((128, 2), tag="pav")
            MMs(pcb[:, 0:1], o1x128r[:], cmeanr[:], start=True, stop=True)
            MMs(pcb[:, 1:2], o1x128r[:], crstd[:], start=True, stop=True)
            yv = sb((128, EC), TR, tag="yv")
            for co in range(EC):
                t_ = sb((128, 1))
                nc.vector.tensor_sub(t_[:], pooled[:, co:co + 1], pcb[:, 0:1])
                nc.vector.tensor_scalar(t_[:], t_[:], pcb[:, 1:2], None, ALU.mult)
                nc.vector.tensor_scalar(yv[:, co:co + 1], t_[:], clng[:, co:co + 1],
                                        clnb[:, co:co + 1], ALU.mult, ALU.add)
            pz = pt((128, 1), tag="pav")
            for co in range(EC):
                MMs(pz[:], cw1T[:, co * 128:(co + 1) * 128], yv[:, co:co + 1],
                   start=(co == 0), stop=(co == EC - 1))
            zv = sb((128, 1), TR)
            nc.vector.tensor_scalar(zv[:], pz[:], cb1[:], None, ALU.add)
            nc.vector.tensor_scalar_max(zv[:].bitcast(TS), zv[:].bitcast(TS), 0.0)
            zv2 = sb((128, 1), TR)
            copy(zv2[:], zv[:].bitcast(TS))
            pout = pt((NCls, 1), tag="pav")
            MMs(pout[:], cw2T[:], zv2[:], start=True, stop=True)
            outv = sb((NCls, 1))
            nc.vector.tensor_scalar(outv[:], pout[:], cb2[:], None, ALU.add)
            nc.sync.dma_start(out_d, outv[:])

    nc.compile()
    return nc


_NC_CACHE = {}


def _host_inputs(inputs):
    I = {k: np.asarray(v, F32) for k, v in inputs.items()}
    h = {}
    h["embw1T"] = np.ascontiguousarray(I["emb_w1"].T).astype(B16)            # [5,128]
    h["embb1"] = I["emb_b1"].reshape(128, 1)

    def wT(w):
        O, II = w.shape
        return np.ascontiguousarray(w.T.reshape(II // 128, 128, O).transpose(1, 0, 2)).reshape(128, -1)

    h["embw2T"] = wT(I["emb_w2"]).astype(B16)                                # [128, 256]
    h["embb2"] = np.ascontiguousarray(I["emb_b2"].reshape(EC, 128).T)
    h["embln"] = np.concatenate([I["emb_ln_g"].reshape(EC, 128).T,
                                 I["emb_ln_b"].reshape(EC, 128).T], axis=1)
    posT = np.zeros((128, EC * SP), F32)
    pe = I["pos_enc"][:S]                                                    # [1000, 256]
    for co in range(EC):
        posT[:, co * SP: co * SP + S] = pe[:, co * 128:(co + 1) * 128].T
    h["posT"] = posT.astype(B16)
    for nm_, key in (("qwT", "qw"), ("kwT", "kw"), ("vwT", "vw"), ("owT", "ow")):
        h[nm_] = np.concatenate([wT(I[key][l]) for l in range(L)], axis=1).astype(B16)
    h["qkvb"] = np.concatenate(
        [np.concatenate([I["qb"][l].reshape(EC, 128).T, I["kb"][l].reshape(EC, 128).T,
                         I["vb"][l].reshape(EC, 128).T], axis=1) for l in range(L)], axis=1)
    h["obias"] = np.concatenate([I["ob"][l].reshape(EC, 128).T for l in range(L)], axis=1)
    h["vbrow"] = I["vb"].reshape(1, L * E).astype(B16)
    h["fw1T"] = np.concatenate([wT(I["f_w1"][l]) for l in range(L)], axis=1).astype(B16)
    h["fw2T"] = np.concatenate([wT(I["f_w2"][l]) for l in range(L)], axis=1).astype(B16)
    h["fb1"] = np.concatenate([I["f_b1"][l].reshape(HC, 128).T for l in range(L)], axis=1)
    h["fb2"] = np.concatenate([I["f_b2"][l].reshape(EC, 128).T for l in range(L)], axis=1)
    h["lng"] = np.concatenate([I["ln_g"][l].reshape(EC, 128).T for l in range(L)], axis=1)
    h["lnb"] = np.concatenate([I["ln_b"][l].reshape(EC, 128).T for l in range(L)], axis=1)
    h["pew1T"] = np.concatenate([np.ascontiguousarray(I["pe_w1"][l].T) for l in range(L)], axis=1)
    h["peb1"] = np.stack([I["pe_b1"][l] for l in range(L)], axis=1)
    h["pew2T"] = np.concatenate([wT(I["pe_w2"][l]) for l in range(L)], axis=1)
    h["peb2"] = np.concatenate([I["pe_b2"][l].reshape(EC, 128).T for l in range(L)], axis=1)
    h["clng"] = np.ascontiguousarray(I["c_ln_g"].reshape(EC, 128).T)
    h["clnb"] = np.ascontiguousarray(I["c_ln_b"].reshape(EC, 128).T)
    h["cw1T"] = wT(I["c_w1"])
    h["cb1"] = I["c_b1"].reshape(128, 1)
    h["cw2T"] = np.ascontiguousarray(I["c_w2"].T)                            # [128, 2]
    h["cb2"] = I["c_b2"].reshape(NCls, 1)
    h["ph_law"] = I["ph_law"].reshape(1, 2); h["ph_lab"] = I["ph_lab"].reshape(1, 1)
    h["ph_fw"] = I["ph_fw"].reshape(1, 1); h["ph_db"] = I["ph_db"].reshape(1, 1)
    tc_ = (np.arange(8)[None, :] * 128 + np.arange(128)[:, None]).astype(F32)
    h["tconst"] = tc_
    h["padneg"] = np.where(tc_ < S, F32(0), F32(-3e38)).astype(F32)
    h["vmask"] = (tc_ < S).astype(F32)
    h["iota50"] = np.broadcast_to(np.arange(50, dtype=F32), (128, 50)).copy()
    h["I50"] = np.eye(50, dtype=F32); h["maskD50"] = np.eye(50, dtype=F32)
    h["I4"] = np.eye(4, dtype=F32); h["I128"] = np.eye(128, dtype=F32)
    h["ones128"] = np.ones((128, 1), F32); h["ones50"] = np.ones((50, 1), F32)
    h["ones1x128"] = np.ones((1, 128), F32); h["ones1x50"] = np.ones((1, 50), F32)
    h["ones4"] = np.ones((4, 1), F32)
    h["onesEC"] = np.ones((EC, 1), F32)
    h["ones128b"] = np.ones((128, 1), B16); h["ones1x128b"] = np.ones((1, 128), B16)
    h["ones1x32b"] = np.ones((1, 32), B16)
    sel8 = np.zeros((128, 256), B16)
    for hh in range(8):
        sel8[32 * (hh % 4), hh * 32:(hh + 1) * 32] = 1
    h["sel8"] = sel8
    h["pwrT"] = np.ones((1, 128), B16)
    v0 = np.full((50, 1), 0.1414, F32); v0[::2, 0] *= -1
    h["v0"] = v0
    h["W0"] = (np.random.default_rng(1234).standard_normal((50, 4)).astype(F32) * F32(0.14))
    h["zeros128"] = np.zeros((128, 256), F32)
    h["epsb"] = np.full((128, 1), 1e-5, F32)
    vm = (tc_ < S).astype(F32)
    h["vmask8"] = np.repeat(vm, 8, axis=1)
    return h


def kernel(**inputs):
    if "nc" not in _NC_CACHE:
        _NC_CACHE["nc"] = build_nc()
    nc = _NC_CACHE["nc"]
    h = _host_inputs(inputs)
    seqs = np.asarray(inputs["sequences"], F32)
    in_maps = []
    for b in range(4):
        m = dict(h)
        seqp = np.zeros((SP, 5), F32)
        seqp[:S] = seqs[b]
        m["seqT5"] = np.ascontiguousarray(seqp.T).astype(B16)
        m["seqPH"] = np.ascontiguousarray(
            seqp.reshape(8, 128, 5).transpose(1, 0, 2).reshape(128, 40))
        in_maps.append(m)
    res = bass_utils.run_bass_kernel_spmd(nc, in_maps, core_ids=[0, 1, 2, 3])
    out = np.stack([res.results[b]["out"][:, 0] for b in range(4)], axis=0)
    return out.astype(np.float32)
